# revision 11
# baseline (speedup 1.0000x reference)
"""MultiBox loss kernel for Trainium2 (Bass/Tile).

Layout: per core, one sample n. Priors padded 8732 -> 8832 = 128*69.
Prior p lives at (partition q = p // 69, column i = p % 69).
Dense tiles are (128, 1380) "i-major": free index i*20 + c.
Box-broadcast tiles are (128, 320) m-major: free index m*20 + c.

Match score: d'' = max(ln(inter) - ln(S) + SIG, 2.0) with S = areaA+areaB,
all values in [2, 4) (one binade -> exact ulp-code packing, no NaN risk).
One fused DVE op (PACKQ) masks the low 11 mantissa bits and ORs in a
(column, m) code: bits 4..10 = 16*(68-i), bits 0..3 = 15-m (via the
PageIdx subdim scan, step = -1 ulp).  iou >= 0.5  <=>  d'' >= SIG+ln(1/3).
"""
import numpy as np

import concourse.bass as bass
import concourse.mybir as mybir
from concourse import tile
from concourse.alu_op_type import AluOpType
from concourse.bass import IndirectOffsetOnAxis

# ---------------- constants ----------------
C, P, M = 20, 8732, 16
QP, I = 128, 69           # partitions x columns
PP = QP * I               # 8832
CM = C * M                # 320
IC = I * C                # 1380
NEG_POS_RATIO = 3.0
SIG = 4.6                 # score shift: d'' = d + SIG clamped at 2.0
ESIG = float(np.exp(-SIG))
# threshold: iou>=0.5 <=> d >= ln(1/3); packed compare on 11-bit-masked floats
_thr = np.float32(np.float32(np.log(np.float32(1.0 / 3.0))) + np.float32(SIG))
THRP = float(np.int32(int(_thr.view(np.int32)) & ~0x7FF).view(np.float32))
ULP22 = float(np.float32(2.0 ** -22))   # 1 ulp in [2,4)
SEL_ROWS, SEL_F = 80, 2208   # selection layout: 4 partitions x (69*32) per class
BISECT_ITERS = 13
DUMP_OFF = 10_000_000     # out-of-bounds scatter offset (dropped)
LN_MIN, LN_RANGE = -15.2, 9.3   # range of 5*ln(w) for box sizes

F32 = mybir.dt.float32
I32 = mybir.dt.int32
AF = mybir.ActivationFunctionType
AX = mybir.AxisListType

# ---------------- custom DVE ops ----------------
_REGISTERED = {}


def _register_op(name, spec, subdim=False):
    if name in _REGISTERED:
        return _REGISTERED[name]
    from concourse.dve_ops import DveOp, OPS, CUSTOM_DVE_SPECS, _SUB_OPCODE_FOR_NAME, _CUSTOM_DVE_ROW_BASE
    from concourse.dve_spec import lower, _has_src1
    from concourse.dve_uop import DveOpSpec
    row = _CUSTOM_DVE_ROW_BASE + len(OPS)
    assert row < 0x20
    _SUB_OPCODE_FOR_NAME[name] = row
    shas = {}
    for ver in ("v3", "v4"):
        s = DveOpSpec(name=name, opcode=row, uops=lower(spec, ver=ver), rd1_en=_has_src1(spec))
        shas[ver] = s.sha(ver)
    op = DveOp(name, spec, subdim=subdim, uops_sha=shas)
    OPS.append(op)
    CUSTOM_DVE_SPECS[name] = spec
    _REGISTERED[name] = op
    return op


def get_ops():
    from concourse.dve_spec import (Spec, Src0, Src1, C0, C1, C2, Zero, One,
                                    maxx, minn, select, eq, Bin, AluOp, PageIdx, Idx)

    ovl = _register_op("ANT_OVL", Spec(
        body=maxx(minn(Src0, C0) - maxx(Src1, C1), C2),
        reference=lambda in0, in1, s0, s1, imm2: np.maximum(
            np.minimum(in0, s0) - np.maximum(in1, s1), imm2).astype(np.float32),
    ))

    # packq: out = ((Src0 - Src1) [bitand] 0xFFFFF800) [bitor] pagecode
    # mask built in-datapath as C1 ^ MaxNeg (scalar loads canonicalize NaN,
    # so 0xFFFFF800 cannot be loaded directly; 0x008007FF ^ 0xFF7FFFFF works).
    # pagecode = bits(s0) - m  (one-ulp steps within the [2,4) binade)
    _pg = PageIdx(C0, C2)
    from concourse.dve_spec import MaxNeg
    # XOR(MaxNeg, Zero) hoists to a latch read at depth 1; One+One latches at
    # depth 2 (read by _code) — distinct swap-init stages, no collision.
    _mask = Bin(AluOp.BITWISE_XOR, C1, Bin(AluOp.BITWISE_XOR, MaxNeg, Zero))
    _d = Src0 - Src1
    _code = Bin(AluOp.BITWISE_XOR, _pg, One + One)   # strip the 0x40000000 binade bit
    packq_body = Bin(AluOp.BITWISE_OR, Bin(AluOp.BITWISE_AND, _d, _mask), _code)

    def _packq_ref(in0, in1, s0, s1, imm2):
        sub = int(np.prod(in0.shape[1:-1]))
        a = in0.reshape((in0.shape[0], sub, in0.shape[-1])).astype(np.float32)
        b = in1.reshape(a.shape).astype(np.float32)
        d = (a - b).astype(np.float32)
        di = d.view(np.int32)
        s1i = int(np.asarray(s1, np.float32).view(np.uint32).flat[0]) ^ 0xFF7FFFFF
        pv = np.asarray(s0, np.float32).reshape(-1, 1, 1) + (
            np.arange(sub, dtype=np.float32).reshape(1, -1, 1) * np.float32(imm2))
        pvi = pv.astype(np.float32).view(np.int32)
        out = (di & np.int32(np.uint32(s1i & 0xFFFFFFFF).astype(np.int64) - (1 << 32) if s1i >= 2**31 else s1i)) | pvi
        return out.view(np.float32).reshape(in0.shape)

    packq = _register_op("ANT_PACKQ", Spec(
        body=packq_body,
        reference=_packq_ref,
    ), subdim=True)

    def _idxmax_ref(in0, in1, s0, s1, imm2):
        n = in0.shape[1]
        out = np.where(in0 >= s0, s1 - np.arange(n)[None, :], 0.0).astype(np.float32)
        return out, out.max(axis=1, keepdims=True)

    idxmax = _register_op("ANT_IDXMAX", Spec(
        body=select(Src0 >= C0, C1 - Idx, Zero),
        accum=AluOp.MAX,
        reference=_idxmax_ref,
    ))

    def _selmax_ref(in0, in1, s0, s1, imm2):
        out = np.where(in0 >= s0, in1, 0.0).astype(np.float32)
        return out, out.max(axis=1, keepdims=True)

    selmax = _register_op("ANT_SELMAX", Spec(
        body=select(Src0 >= C0, Src1, Zero),
        accum=AluOp.MAX,
        reference=_selmax_ref,
    ))

    def _selv_ref(in0, in1, s0, s1, imm2):
        return np.where(in0 == s0, in1, 0.0).astype(np.float32)

    selv = _register_op("ANT_SELV", Spec(
        body=select(eq(Src0, C0), Src1, Zero),
        reference=_selv_ref,
    ))

    def _sumgt_ref(in0, in1, s0, s1, imm2):
        out = np.where(in0 > s0, in0, 0.0).astype(np.float32)
        return out, out.sum(axis=1, keepdims=True, dtype=np.float32)

    sumgt = _register_op("ANT_SUMGT", Spec(
        body=select(Src0 > C0, Src0, Zero),
        accum=AluOp.ADD,
        reference=_sumgt_ref,
    ))
    return ovl, packq, idxmax, selmax, sumgt, selv


# ---------------- host-side input prep ----------------
def prep_core_inputs(scores_nc, locs_nc, boxes_nc):
    sc = np.zeros((C, QP * 138), np.float32)
    sc[:, : P * 2] = scores_nc.reshape(C, P * 2)
    lc = np.zeros((C, QP * 276), np.float32)
    lc[:, : P * 4] = locs_nc.reshape(C, P * 4)
    # boxes m-major: free index m*C + c
    bt = np.ascontiguousarray(boxes_nc.transpose(1, 0, 2)).reshape(1, CM * 4)
    return {
        "scores_pad": sc,
        "locs_pad": lc,
        "boxes_t": bt.astype(np.float32),
    }


def prep_shared_inputs(priors):
    pr = np.zeros((PP, 4), np.float32)
    pr[:P] = priors
    pr[P:, 0] = 50.0 + np.arange(PP - P)
    pr[P:, 1] = 50.0
    pr[P:, 2] = 0.01
    pr[P:, 3] = 0.01

    ident = np.eye(QP, dtype=np.float32)
    ind120 = np.zeros((SEL_ROWS, C), np.float32)
    for k in range(SEL_ROWS):
        ind120[k, k // 4] = 1.0
    indT = np.ascontiguousarray(ind120.T)
    pidx = np.arange(QP)[:, None] * I + np.arange(I)[None, :]   # (128, 69)
    padmask = (pidx < P).astype(np.float32)[:, :, None].repeat(C, 2).reshape(QP, IC)
    part = np.arange(QP)
    # m-major cm mapping: c = cm % 20, m = cm // 20
    coffs = np.stack([((b * QP + part) % C).astype(np.float32) for b in range(3)], 1)
    mvals = np.stack([(15.0 - (b * QP + part) // C).astype(np.float32) for b in range(3)], 1)
    # cross-block dedup mask: laterm[part, b*CM + cm2] = 1 if same class, later m
    cmall = np.arange(CM)
    laterm = np.zeros((QP, 3 * CM), np.float32)
    for b in range(3):
        cm1 = b * QP + part  # (128,)
        same_c = (cm1[:, None] % C) == (cmall[None, :] % C)
        later_m = (cmall[None, :] // C) > (cm1[:, None] // C)
        laterm[:, b * CM:(b + 1) * CM] = (same_c & later_m & (cm1 < CM)[:, None]).astype(np.float32)
    return {
        "priors_t": pr,
        "ident": ident,
        "ind120": ind120,
        "indT": indT,
        "laterm": laterm,
        "coffs": coffs,
        "mvals": mvals,
        "padmask": padmask,
    }


# ---------------- the kernel ----------------
def build_kernel(tc, outs, ins):
    nc = tc.nc
    OVL, PACKQ, IDXMAX, SELMAX, SUMGT, SELV = get_ops()

    out_part = outs["part"]      # (8, 20) f32

    from contextlib import ExitStack
    with ExitStack() as ctx:
        cpool = ctx.enter_context(tc.tile_pool(name="const", bufs=1))
        lpool = ctx.enter_context(tc.tile_pool(name="loop", bufs=2))
        ppool = ctx.enter_context(tc.tile_pool(name="psum", bufs=2, space="PSUM"))
        dpool = ctx.enter_context(tc.tile_pool(name="dram", bufs=1, space="DRAM"))
        _build(nc, tc, cpool, lpool, ppool, dpool, ins, out_part,
               OVL, PACKQ, IDXMAX, SELMAX, SUMGT, SELV)


def _build(nc, tc, cpool, lpool, ppool, dpool, ins, out_part, OVL, PACKQ, IDXMAX, SELMAX, SUMGT, SELV):
    scores = ins["scores_pad"]
    locs = ins["locs_pad"]
    boxes_t = ins["boxes_t"]
    priors_t = ins["priors_t"]

    # ---- load constants / inputs ----
    PR = cpool.tile([QP, I, 4], F32)
    nc.sync.dma_start(out=PR[:], in_=priors_t.rearrange("(q i) k -> q i k", q=QP))
    IDENT = cpool.tile([QP, QP], F32)
    nc.sync.dma_start(out=IDENT[:], in_=ins["ident"])
    IND120 = cpool.tile([SEL_ROWS, C], F32)
    nc.sync.dma_start(out=IND120[:], in_=ins["ind120"])
    INDT = cpool.tile([C, SEL_ROWS], F32)
    nc.sync.dma_start(out=INDT[:], in_=ins["indT"])
    LATERM = cpool.tile([QP, 3 * CM], F32)
    nc.sync.dma_start(out=LATERM[:], in_=ins["laterm"])

    SC = cpool.tile([QP, C, 138], F32)
    nc.sync.dma_start(out=SC[:], in_=scores.rearrange("c (q e) -> q c e", q=QP))
    PL = cpool.tile([QP, C, 276], F32)
    nc.sync.dma_start(out=PL[:], in_=locs.rearrange("c (q e) -> q c e", q=QP))
    BT = cpool.tile([1, CM * 4], F32)
    nc.sync.dma_start(out=BT[:], in_=boxes_t)

    PADM = cpool.tile([QP, IC], F32, tag="padm")
    nc.sync.dma_start(out=PADM[:], in_=ins["padmask"])
    CONSTI = cpool.tile([QP, 9], I32)
    # 0: pack-mask xor-seed 0x008007FF (^-FLT_MAX -> 0xFFFFF800),
    # 1: col extract 0x7F0, 2: m extract 0xF,
    # 3: 63, 4: 0xFC0, 5: 0x3F000, 6: 0xFC0000, 7: unused, 8: 0
    for _k, _v in enumerate([0x008007FF, 0x7F0, 0xF, 63, 0xFC0, 0x3F000, 0xFC0000, 0, 0]):
        nc.vector.memset(CONSTI[:, _k:_k + 1], _v)
    # per-column code base: bits 0x40000000 + 16*(68-i) + 15
    COLB = cpool.tile([QP, I], I32)
    nc.gpsimd.iota(COLB[:], pattern=[[-16, I]], base=0x40000000 + 16 * 68 + 15,
                   channel_multiplier=0)
    COFF = cpool.tile([QP, 3], F32)
    nc.sync.dma_start(out=COFF[:], in_=ins["coffs"])
    VALS = cpool.tile([QP, 3], F32)
    nc.sync.dma_start(out=VALS[:], in_=ins["mvals"])

    # ---- prior-derived tiles (128, 69) ----
    pcx = PR[:, :, 0]
    pcy = PR[:, :, 1]
    pw = PR[:, :, 2]
    ph = PR[:, :, 3]
    PX1 = cpool.tile([QP, I], F32)
    PX2 = cpool.tile([QP, I], F32)
    PY1 = cpool.tile([QP, I], F32)
    PY2 = cpool.tile([QP, I], F32)
    PAREA = cpool.tile([QP, I], F32)   # prior area pre-scaled by exp(-SIG)
    nc.vector.scalar_tensor_tensor(out=PX1[:], in0=pw, scalar=-0.5, in1=pcx,
                                   op0=AluOpType.mult, op1=AluOpType.add)
    nc.vector.scalar_tensor_tensor(out=PX2[:], in0=pw, scalar=0.5, in1=pcx,
                                   op0=AluOpType.mult, op1=AluOpType.add)
    nc.vector.scalar_tensor_tensor(out=PY1[:], in0=ph, scalar=-0.5, in1=pcy,
                                   op0=AluOpType.mult, op1=AluOpType.add)
    nc.vector.scalar_tensor_tensor(out=PY2[:], in0=ph, scalar=0.5, in1=pcy,
                                   op0=AluOpType.mult, op1=AluOpType.add)
    nc.vector.scalar_tensor_tensor(out=PAREA[:], in0=pw, scalar=ESIG, in1=ph,
                                   op0=AluOpType.mult, op1=AluOpType.mult)
    IPW = cpool.tile([QP, I], F32)   # 10 / pw
    IPH = cpool.tile([QP, I], F32)
    scr69 = cpool.tile([QP, I], F32)
    nc.vector.reciprocal_approx_accurate(out=IPW[:], in_=pw, scratch=scr69[:])
    nc.vector.reciprocal_approx_accurate(out=IPH[:], in_=ph, scratch=scr69[:])
    nc.vector.tensor_scalar_mul(IPW[:], IPW[:], 10.0)
    nc.vector.tensor_scalar_mul(IPH[:], IPH[:], 10.0)
    LPW5 = cpool.tile([QP, I], F32)  # 5*ln(pw)
    LPH5 = cpool.tile([QP, I], F32)
    nc.scalar.activation(out=LPW5[:], in_=pw, func=AF.Ln)
    nc.scalar.activation(out=LPH5[:], in_=ph, func=AF.Ln)
    nc.vector.tensor_scalar_mul(LPW5[:], LPW5[:], 5.0)
    nc.vector.tensor_scalar_mul(LPH5[:], LPH5[:], 5.0)

    # per-prior l1 helper tiles (no loop deps; emitted early for overlap)
    IPW63 = cpool.tile([QP, I], F32)
    IPH63 = cpool.tile([QP, I], F32)
    nc.vector.tensor_scalar_mul(IPW63[:], IPW[:], 1.0 / 63.0)
    nc.vector.tensor_scalar_mul(IPH63[:], IPH[:], 1.0 / (63.0 * 64.0))
    PCXI = cpool.tile([QP, I], F32)
    PCYI = cpool.tile([QP, I], F32)
    nc.vector.tensor_tensor(out=PCXI[:], in0=pcx, in1=IPW[:], op=AluOpType.mult)
    nc.vector.tensor_tensor(out=PCYI[:], in0=pcy, in1=IPH[:], op=AluOpType.mult)

    # ---- box-derived broadcast tiles (128, 320) m-major ----
    bx1v = BT[:, 0::4]
    by1v = BT[:, 1::4]
    bx2v = BT[:, 2::4]
    by2v = BT[:, 3::4]
    BD = cpool.tile([1, CM * 10], F32, tag="cbslot")
    s = [BD[:, k * CM:(k + 1) * CM] for k in range(10)]
    # 0 bx1, 1 bx2, 2 by1, 3 by2, 4 bcx, 5 bcy, 6 areab*e^-SIG, 7 lnbw5, 8 lnbh5, 9 quad
    nc.vector.tensor_copy(out=s[0], in_=bx1v)
    nc.vector.tensor_copy(out=s[1], in_=bx2v)
    nc.vector.tensor_copy(out=s[2], in_=by1v)
    nc.vector.tensor_copy(out=s[3], in_=by2v)
    t1 = cpool.tile([1, CM], F32)
    nc.vector.tensor_tensor(out=t1[:], in0=bx1v, in1=bx2v, op=AluOpType.add)
    nc.vector.tensor_scalar_mul(s[4], t1[:], 0.5)
    nc.vector.tensor_tensor(out=t1[:], in0=by1v, in1=by2v, op=AluOpType.add)
    nc.vector.tensor_scalar_mul(s[5], t1[:], 0.5)
    tbw = cpool.tile([1, CM], F32)
    tbh = cpool.tile([1, CM], F32)
    nc.vector.tensor_tensor(out=tbw[:], in0=bx2v, in1=bx1v, op=AluOpType.subtract)
    nc.vector.tensor_tensor(out=tbh[:], in0=by2v, in1=by1v, op=AluOpType.subtract)
    nc.vector.scalar_tensor_tensor(out=s[6], in0=tbw[:], scalar=ESIG, in1=tbh[:],
                                   op0=AluOpType.mult, op1=AluOpType.mult)
    nc.scalar.activation(out=s[7], in_=tbw[:], func=AF.Ln)
    nc.scalar.activation(out=s[8], in_=tbh[:], func=AF.Ln)
    nc.vector.tensor_scalar_mul(s[7], s[7], 5.0)
    nc.vector.tensor_scalar_mul(s[8], s[8], 5.0)
    # quad code: round(bcx*63) + 64*round(bcy*63) + 64^2*round((lnw5-LN_MIN)*63/LN_RANGE) + 64^3*(...)
    enc = cpool.tile([1, CM], I32)
    encf = cpool.tile([1, CM], F32)
    quad = s[9]
    nc.vector.tensor_scalar(out=enc[:], in0=s[4], scalar1=63.0, scalar2=0.5,
                            op0=AluOpType.mult, op1=AluOpType.add)
    nc.vector.tensor_copy(out=quad, in_=enc[:])
    nc.vector.tensor_scalar(out=enc[:], in0=s[5], scalar1=63.0, scalar2=0.5,
                            op0=AluOpType.mult, op1=AluOpType.add)
    nc.vector.tensor_copy(out=encf[:], in_=enc[:])
    nc.vector.scalar_tensor_tensor(out=quad, in0=encf[:], scalar=64.0, in1=quad,
                                   op0=AluOpType.mult, op1=AluOpType.add)
    nc.vector.tensor_scalar(out=enc[:], in0=s[7], scalar1=63.0 / LN_RANGE,
                            scalar2=-LN_MIN * 63.0 / LN_RANGE + 0.5,
                            op0=AluOpType.mult, op1=AluOpType.add)
    nc.vector.tensor_copy(out=encf[:], in_=enc[:])
    nc.vector.scalar_tensor_tensor(out=quad, in0=encf[:], scalar=4096.0, in1=quad,
                                   op0=AluOpType.mult, op1=AluOpType.add)
    nc.vector.tensor_scalar(out=enc[:], in0=s[8], scalar1=63.0 / LN_RANGE,
                            scalar2=-LN_MIN * 63.0 / LN_RANGE + 0.5,
                            op0=AluOpType.mult, op1=AluOpType.add)
    nc.vector.tensor_copy(out=encf[:], in_=enc[:])
    nc.vector.scalar_tensor_tensor(out=quad, in0=encf[:], scalar=262144.0, in1=quad,
                                   op0=AluOpType.mult, op1=AluOpType.add)

    ONES1 = cpool.tile([1, QP], F32)
    nc.vector.memset(ONES1[:], 1.0)
    BB = cpool.tile([QP, CM * 10], F32)
    tot = CM * 10
    off = 0
    while off < tot:
        w = min(512, tot - off)
        pt = ppool.tile([QP, w], F32, tag="bcast")
        nc.tensor.matmul(out=pt[:], lhsT=ONES1[:], rhs=BD[:, off:off + w], start=True, stop=True)
        nc.scalar.copy(out=BB[:, off:off + w], in_=pt[:])
        off += w
    BX1 = BB[:, 0 * CM:1 * CM]
    BX2 = BB[:, 1 * CM:2 * CM]
    BY1 = BB[:, 2 * CM:3 * CM]
    BY2 = BB[:, 3 * CM:4 * CM]
    BAR = BB[:, 6 * CM:7 * CM]
    QUADB = BB[:, 9 * CM:10 * CM]

    # ---- CE (no dependency on matching; emitted early for engine overlap) ----
    DM = cpool.tile([QP, IC], F32, tag="dm")
    sc4 = SC[:].rearrange("p c (i two) -> p c i two", two=2)
    dm3 = DM[:].rearrange("p (i c) -> p i c", c=C)
    nc.vector.tensor_tensor(out=dm3.rearrange("p i c -> p c i"),
                            in0=sc4[:, :, :, 1], in1=sc4[:, :, :, 0], op=AluOpType.subtract)
    CE = cpool.tile([QP, IC], F32)
    nc.scalar.activation(out=CE[:], in_=DM[:], func=AF.Exp)
    nc.scalar.activation(out=CE[:], in_=CE[:], func=AF.Ln, bias=1.0)

    # ---- accumulators ----
    QMM = cpool.tile([QP, I, C], F32)
    QPA = cpool.tile([QP, CM], F32)
    nc.vector.memset(QPA[:], 0.0)

    # ================= main loop over columns i =================
    for i in range(I):
        xov = lpool.tile([QP, CM], F32, tag="xov")
        nc.vector._custom_dve(OVL, out=xov[:], in0=BX2, in1=BX1,
                              s0=PX2[:, i:i + 1], s1=PX1[:, i:i + 1], imm2=1e-18)
        yov = lpool.tile([QP, CM], F32, tag="yov")
        nc.vector._custom_dve(OVL, out=yov[:], in0=BY2, in1=BY1,
                              s0=PY2[:, i:i + 1], s1=PY1[:, i:i + 1], imm2=1e-18)
        inter = lpool.tile([QP, CM], F32, tag="inter")
        nc.vector.tensor_tensor(out=inter[:], in0=xov[:], in1=yov[:], op=AluOpType.mult)
        S = lpool.tile([QP, CM], F32, tag="S")
        nc.scalar.activation(out=S[:], in_=BAR, func=AF.Identity,
                             bias=PAREA[:, i:i + 1], scale=1.0)
        lnI = lpool.tile([QP, CM], F32, tag="lnI")
        nc.scalar.activation(out=lnI[:], in_=inter[:], func=AF.Ln)
        lnS = lpool.tile([QP, CM], F32, tag="lnS")
        nc.scalar.activation(out=lnS[:], in_=S[:], func=AF.Ln)
        qm = lpool.tile([QP, CM], F32, tag="qm")
        nc.vector._custom_dve(
            PACKQ,
            out=qm[:].rearrange("p (m c) -> p m c", m=M),
            in0=lnI[:].rearrange("p (m c) -> p m c", m=M),
            in1=lnS[:],
            s0=COLB[:, i:i + 1].bitcast(F32), s1=CONSTI[:, 0:1].bitcast(F32), imm2=-ULP22)
        nc.vector.tensor_tensor(out=QPA[:], in0=QPA[:], in1=qm[:], op=AluOpType.max)
        nc.vector.tensor_reduce(out=QMM[:, i, :],
                                in_=qm[:].rearrange("p (m c) -> p c m", c=C),
                                axis=AX.X, op=AluOpType.max)

    QMMf = QMM[:].rearrange("p i c -> p (i c)")
    QMMi = QMMf.bitcast(I32)

    # ================= pos mask, m* =================
    POSB = cpool.tile([QP, IC], F32, tag="posb")
    nc.vector.tensor_scalar(out=POSB[:], in0=QMMf, scalar1=THRP, scalar2=0.0,
                            op0=AluOpType.is_ge, op1=AluOpType.max)
    # m-code (15-m) in low 4 bits
    MSI = cpool.tile([QP, IC], I32, tag="ic_int")
    nc.vector.scalar_tensor_tensor(out=MSI[:], in0=QMMi, scalar=CONSTI[:, 2:3],
                                   in1=CONSTI[:, 8:9].to_broadcast([QP, IC]),
                                   op0=AluOpType.bitwise_and, op1=AluOpType.bitwise_or)
    MS = cpool.tile([QP, IC], F32)
    nc.vector.tensor_copy(out=MS[:], in_=MSI[:])

    # ================= prior_for_obj (forced positives) =================
    QPAf = QPA[:]
    PSTARI = cpool.tile([QP, 3], I32)
    PSTB = cpool.tile([QP, 3], F32)
    PSTROW = cpool.tile([1, CM], F32)
    for b in range(3):
        w = 128 if b < 2 else 64
        tp = ppool.tile([QP, QP], F32, tag="ptr")
        nc.tensor.transpose(out=tp[:w, :], in_=QPAf[:, b * QP:b * QP + w], identity=IDENT[:])
        TQ = lpool.tile([QP, QP], F32, tag="TQ")
        nc.scalar.copy(out=TQ[:w, :], in_=tp[:w, :])
        vmax = lpool.tile([QP, 1], F32, tag="vmax")
        nc.vector.tensor_reduce(out=vmax[:w], in_=TQ[:w, :], axis=AX.X, op=AluOpType.max)
        qd = lpool.tile([QP, 1], F32, tag="qd")
        sc1 = lpool.tile([QP, QP], F32, tag="sc1")
        nc.vector._custom_dve(IDXMAX, out=sc1[:w, :], accum_out=qd[:w], in0=TQ[:w, :],
                              s0=vmax[:w], s1=127.0)
        TLI = lpool.tile([QP, QP], I32, tag="TLI")
        nc.vector.scalar_tensor_tensor(out=TLI[:w, :], in0=TQ[:w, :].bitcast(I32),
                                       scalar=CONSTI[:w, 1:2],
                                       in1=CONSTI[:w, 8:9].to_broadcast([w, QP]),
                                       op0=AluOpType.bitwise_and, op1=AluOpType.bitwise_or)
        TLF = lpool.tile([QP, QP], F32, tag="TLF")
        nc.vector.tensor_copy(out=TLF[:w, :], in_=TLI[:w, :])
        colv = lpool.tile([QP, 1], F32, tag="ilow")
        sc2 = lpool.tile([QP, QP], F32, tag="sc2")
        nc.vector._custom_dve(SELMAX, out=sc2[:w, :], accum_out=colv[:w], in0=TQ[:w, :],
                              in1=TLF[:w, :], s0=vmax[:w])
        # p* = (127 - qd)*69 + (68 - colv/16)
        pst = PSTB[:, b:b + 1]
        nc.vector.tensor_scalar(out=pst[:w], in0=qd[:w], scalar1=-69.0,
                                scalar2=float(127 * 69 + 68),
                                op0=AluOpType.mult, op1=AluOpType.add)
        nc.vector.scalar_tensor_tensor(out=pst[:w], in0=colv[:w], scalar=-1.0 / 16.0,
                                       in1=pst[:w], op0=AluOpType.mult, op1=AluOpType.add)
        # gather p* of this block into row layout for cross-block dedup
        tpr = ppool.tile([QP, QP], F32, tag="ptr")
        nc.tensor.transpose(out=tpr[:1, :w], in_=pst[:w], identity=IDENT[:w, :w])
        nc.scalar.copy(out=PSTROW[:, b * QP:b * QP + w], in_=tpr[:1, :w])
    # broadcast all 320 p* values to all partitions
    ptp = ppool.tile([QP, CM], F32, tag="ptall")
    nc.tensor.matmul(out=ptp[:], lhsT=ONES1[:], rhs=PSTROW[:], start=True, stop=True)
    PTALL = cpool.tile([QP, CM], F32)
    nc.scalar.copy(out=PTALL[:], in_=ptp[:])
    for b in range(3):
        w = 128 if b < 2 else 64
        pst = PSTB[:, b:b + 1]
        EQM = lpool.tile([QP, CM], F32, tag="EQM")
        nc.vector.tensor_tensor(out=EQM[:w, :], in0=pst[:w].to_broadcast([w, CM]),
                                in1=PTALL[:w, :], op=AluOpType.is_equal)
        nc.vector.tensor_tensor(out=EQM[:w, :], in0=EQM[:w, :],
                                in1=LATERM[:w, b * CM:(b + 1) * CM], op=AluOpType.mult)
        dom = lpool.tile([QP, 1], F32, tag="dom")
        nc.vector.tensor_reduce(out=dom[:w], in_=EQM[:w, :], axis=AX.X, op=AluOpType.max)
        # offset = p* * 20 + c; dominated -> +DUMP_OFF (dropped by bounds check)
        offf = lpool.tile([QP, 1], F32, tag="offf")
        nc.vector.scalar_tensor_tensor(out=offf[:w], in0=pst[:w], scalar=20.0,
                                       in1=COFF[:w, b:b + 1],
                                       op0=AluOpType.mult, op1=AluOpType.add)
        nc.vector.scalar_tensor_tensor(out=offf[:w], in0=dom[:w], scalar=float(DUMP_OFF),
                                       in1=offf[:w],
                                       op0=AluOpType.mult, op1=AluOpType.add)
        nc.vector.tensor_copy(out=PSTARI[:w, b:b + 1], in_=offf[:w])

    # FMD scratch in DRAM, init -1, scatter (15-m) codes, read back
    FMD = dpool.tile([PP * C, 1], F32)
    NEG1 = cpool.tile([QP, IC], F32, tag="l1a")
    nc.vector.memset(NEG1[:], -1.0)
    nc.sync.dma_start(out=FMD[:].rearrange("(q f) one -> q (f one)", q=QP), in_=NEG1[:])
    for b in range(3):
        w = 128 if b < 2 else 64
        nc.gpsimd.indirect_dma_start(
            out=FMD[:],
            out_offset=IndirectOffsetOnAxis(ap=PSTARI[:w, b:b + 1], axis=0),
            in_=VALS[:w, b:b + 1],
            in_offset=None,
            bounds_check=PP * C - 1,
            oob_is_err=False,
        )
    FM = cpool.tile([QP, IC], F32, tag="fm")
    nc.sync.dma_start(out=FM[:], in_=FMD[:].rearrange("(q f) one -> q (f one)", q=QP))

    FGE = cpool.tile([QP, IC], F32)
    nc.vector.tensor_scalar(out=FGE[:], in0=FM[:], scalar1=0.0, scalar2=0.0,
                            op0=AluOpType.is_ge, op1=AluOpType.max)
    POSB2 = POSB
    nc.vector.tensor_tensor(out=POSB2[:], in0=POSB[:], in1=FGE[:], op=AluOpType.max)
    FGEI = cpool.tile([QP, IC], I32, tag="ic_int")
    nc.vector.tensor_copy(out=FGEI[:], in_=FGE[:])
    MS2 = MS
    nc.vector.copy_predicated(out=MS2[:], mask=FGEI[:], data=FM[:])

    # ================= CE pos/neg splits =================
    CEP = cpool.tile([QP, IC], F32, tag="cep")
    nc.vector.tensor_tensor(out=CEP[:], in0=PADM[:], in1=POSB2[:], op=AluOpType.subtract)
    CEN = cpool.tile([QP, C, I], F32, tag="scbslot")
    cen_im = CEN[:].rearrange("p c i -> p i c")
    nc.vector.tensor_tensor(out=cen_im, in0=CE[:].rearrange("p (i c) -> p i c", c=C),
                            in1=CEP[:].rearrange("p (i c) -> p i c", c=C), op=AluOpType.mult)
    CPT = cpool.tile([QP, IC], F32, tag="gt")
    nc.vector.tensor_tensor(out=CPT[:], in0=CE[:], in1=DM[:], op=AluOpType.subtract)
    nc.vector.tensor_tensor(out=CPT[:], in0=CPT[:], in1=POSB2[:], op=AluOpType.mult)

    # ================= counts / class sums =================
    NPQ = cpool.tile([QP, C], F32)
    nc.vector.tensor_reduce(out=NPQ[:], in_=POSB2[:].rearrange("p (i c) -> p c i", c=C),
                            axis=AX.X, op=AluOpType.add)
    CPQ = cpool.tile([QP, C], F32)
    nc.vector.tensor_reduce(out=CPQ[:], in_=CPT[:].rearrange("p (i c) -> p c i", c=C),
                            axis=AX.X, op=AluOpType.add)
    ONESC = cpool.tile([QP, 1], F32)
    nc.vector.memset(ONESC[:], 1.0)
    NPC_p = ppool.tile([1, C], F32, tag="pmm")
    nc.tensor.matmul(out=NPC_p[:], lhsT=ONESC[:], rhs=NPQ[:], start=True, stop=True)
    CPC_p = ppool.tile([1, C], F32, tag="pmm")
    nc.tensor.matmul(out=CPC_p[:], lhsT=ONESC[:], rhs=CPQ[:], start=True, stop=True)
    NPC = cpool.tile([1, C], F32)
    nc.scalar.copy(out=NPC[:], in_=NPC_p[:])
    CPC = cpool.tile([1, C], F32)
    nc.scalar.copy(out=CPC[:], in_=CPC_p[:])

    kp = ppool.tile([C, 1], F32, tag="pmm")
    nc.tensor.transpose(out=kp[:], in_=NPC[:], identity=IDENT[:1, :1])
    KC = cpool.tile([C, 1], F32)
    nc.scalar.copy(out=KC[:], in_=kp[:])
    nc.vector.tensor_scalar_mul(KC[:], KC[:], NEG_POS_RATIO)

    # ================= hard-negative selection =================
    CB = cpool.tile([SEL_ROWS, SEL_F], F32, tag="cbslot")
    for c in range(C):
        nc.sync.dma_start(out=CB[c * 4:(c + 1) * 4, :], in_=CEN[:, c, :])

    LO = cpool.tile([C, 1], F32)
    HI = cpool.tile([C, 1], F32)
    TC_ = cpool.tile([C, 1], F32)
    nc.vector.memset(LO[:], 0.8)
    nc.vector.memset(HI[:], 4.0)
    T120 = cpool.tile([SEL_ROWS, 1], F32)
    CNT6 = cpool.tile([SEL_ROWS, 1], F32)
    CNTC = cpool.tile([C, 1], F32)
    scb = cpool.tile([SEL_ROWS, SEL_F], F32, tag="scbslot")
    for it in range(BISECT_ITERS):
        nc.vector.tensor_tensor(out=TC_[:], in0=LO[:], in1=HI[:], op=AluOpType.add)
        nc.vector.tensor_scalar_mul(TC_[:], TC_[:], 0.5)
        tp120 = ppool.tile([SEL_ROWS, 1], F32, tag="pmm")
        nc.tensor.matmul(out=tp120[:], lhsT=INDT[:], rhs=TC_[:], start=True, stop=True)
        nc.scalar.copy(out=T120[:], in_=tp120[:])
        nc.vector.tensor_scalar(out=scb[:], in0=CB[:], scalar1=T120[:, :1], scalar2=0.0,
                                op0=AluOpType.is_gt, op1=AluOpType.add, accum_out=CNT6[:])
        tpc = ppool.tile([C, 1], F32, tag="pmm")
        nc.tensor.matmul(out=tpc[:], lhsT=IND120[:], rhs=CNT6[:], start=True, stop=True)
        nc.scalar.copy(out=CNTC[:], in_=tpc[:])
        gm = lpool.tile([C, 1], I32, tag="gm")
        nc.vector.tensor_tensor(out=gm[:], in0=CNTC[:], in1=KC[:], op=AluOpType.is_ge)
        nc.vector.copy_predicated(out=LO[:], mask=gm[:], data=TC_[:])
        lm = lpool.tile([C, 1], I32, tag="lm")
        nc.vector.tensor_tensor(out=lm[:], in0=CNTC[:], in1=KC[:], op=AluOpType.is_lt)
        nc.vector.copy_predicated(out=HI[:], mask=lm[:], data=TC_[:])
    tp120 = ppool.tile([SEL_ROWS, 1], F32, tag="pmm")
    nc.tensor.matmul(out=tp120[:], lhsT=INDT[:], rhs=LO[:], start=True, stop=True)
    nc.scalar.copy(out=T120[:], in_=tp120[:])
    SUM6 = cpool.tile([SEL_ROWS, 1], F32)
    nc.vector._custom_dve(SUMGT, out=scb[:], accum_out=SUM6[:], in0=CB[:], s0=T120[:, :1])
    nc.vector.tensor_scalar(out=scb[:], in0=CB[:], scalar1=T120[:, :1], scalar2=0.0,
                            op0=AluOpType.is_gt, op1=AluOpType.add, accum_out=CNT6[:])
    SUMC_p = ppool.tile([C, 1], F32, tag="pmm")
    nc.tensor.matmul(out=SUMC_p[:], lhsT=IND120[:], rhs=SUM6[:], start=True, stop=True)
    CNTC_p = ppool.tile([C, 1], F32, tag="pmm")
    nc.tensor.matmul(out=CNTC_p[:], lhsT=IND120[:], rhs=CNT6[:], start=True, stop=True)
    CH = cpool.tile([C, 1], F32)
    nc.scalar.copy(out=CNTC[:], in_=CNTC_p[:])
    nc.vector.tensor_tensor(out=CH[:], in0=KC[:], in1=CNTC[:], op=AluOpType.subtract)
    nc.vector.tensor_tensor(out=CH[:], in0=CH[:], in1=LO[:], op=AluOpType.mult)
    SUMC = cpool.tile([C, 1], F32)
    nc.scalar.copy(out=SUMC[:], in_=SUMC_p[:])
    nc.vector.tensor_tensor(out=CH[:], in0=CH[:], in1=SUMC[:], op=AluOpType.add)

    # ================= localization loss =================
    # 16-way select of quad-encoded box quantities by m* code (15-m)
    G = cpool.tile([QP, IC], F32, tag="gt")
    g3 = G[:].rearrange("p (i c) -> p i c", c=C)

    def quadview(m):
        return QUADB[:, m * C:(m + 1) * C].unsqueeze(1).to_broadcast([QP, I, C])

    nc.vector.memset(G[:], 0.0)
    ms3 = MS2[:].rearrange("p (i c) -> p i c", c=C)
    TQM = cpool.tile([QP, I, C], F32, tag="tqm")
    tq3 = TQM[:]
    for m in range(M):
        nc.vector._custom_dve(SELV, out=tq3, in0=ms3, in1=quadview(m), s0=float(15 - m))
        nc.vector.tensor_tensor(out=g3, in0=tq3, in1=g3, op=AluOpType.max)
    GI = cpool.tile([QP, IC], I32, tag="ic_int")
    nc.vector.tensor_copy(out=GI[:], in_=G[:])

    L1A = cpool.tile([QP, IC], F32, tag="l1a")
    nc.vector.memset(L1A[:], 0.0)
    EC = cpool.tile([QP, IC], F32, tag="dm")
    ECI = cpool.tile([QP, IC], I32, tag="ec_int")
    TM2 = cpool.tile([QP, IC], F32, tag="cep")
    tm3 = TM2[:].rearrange("p (i c) -> p i c", c=C)
    ec3 = EC[:].rearrange("p (i c) -> p i c", c=C)
    pl5 = PL[:].rearrange("p c (i four) -> p c i four", four=4)

    def bc69(t):
        return t[:].unsqueeze(2).to_broadcast([QP, I, C])

    def l1_xy(mask_col, scale_t, pci_t, k_coord):
        nc.vector.scalar_tensor_tensor(out=ECI[:], in0=GI[:], scalar=CONSTI[:, mask_col:mask_col + 1],
                                       in1=CONSTI[:, 8:9].to_broadcast([QP, IC]),
                                       op0=AluOpType.bitwise_and, op1=AluOpType.bitwise_or)
        nc.vector.tensor_copy(out=EC[:], in_=ECI[:])
        # A = pl + pcx*ipw ; t = e * (ipw/63/shift); diff = A - t
        plv = pl5[:, :, :, k_coord].rearrange("p c i -> p i c")
        nc.vector.tensor_tensor(out=tm3, in0=plv, in1=bc69(pci_t), op=AluOpType.add)
        nc.vector.tensor_tensor(out=ec3, in0=ec3, in1=bc69(scale_t), op=AluOpType.mult)
        nc.vector.tensor_tensor(out=tm3, in0=tm3, in1=ec3, op=AluOpType.subtract)
        nc.vector.scalar_tensor_tensor(out=TM2[:], in0=TM2[:], scalar=-1.0, in1=TM2[:],
                                       op0=AluOpType.mult, op1=AluOpType.max)
        nc.vector.tensor_tensor(out=L1A[:], in0=L1A[:], in1=TM2[:], op=AluOpType.add)

    l1_xy(3, IPW63, PCXI, 0)          # cx: e in [0,63], value e/63 * ipw
    l1_xy(4, IPH63, PCYI, 1)          # cy: e-bits at <<6; scale = iph/(63*64)

    # w/h coords: A = pl + lnpw5 - LN_MIN ; t = e * (LN_RANGE/63/shift)
    def l1_wh(mask_col, shift, lp5, k_coord):
        nc.vector.scalar_tensor_tensor(out=ECI[:], in0=GI[:], scalar=CONSTI[:, mask_col:mask_col + 1],
                                       in1=CONSTI[:, 8:9].to_broadcast([QP, IC]),
                                       op0=AluOpType.bitwise_and, op1=AluOpType.bitwise_or)
        nc.vector.tensor_copy(out=EC[:], in_=ECI[:])
        plv = pl5[:, :, :, k_coord].rearrange("p c i -> p i c")
        nc.vector.scalar_tensor_tensor(out=tm3, in0=plv, scalar=-LN_MIN, in1=bc69(lp5),
                                       op0=AluOpType.add, op1=AluOpType.add)
        nc.vector.tensor_scalar_mul(EC[:], EC[:], LN_RANGE / 63.0 / shift)
        nc.vector.tensor_tensor(out=tm3, in0=tm3, in1=ec3, op=AluOpType.subtract)
        nc.vector.scalar_tensor_tensor(out=TM2[:], in0=TM2[:], scalar=-1.0, in1=TM2[:],
                                       op0=AluOpType.mult, op1=AluOpType.max)
        nc.vector.tensor_tensor(out=L1A[:], in0=L1A[:], in1=TM2[:], op=AluOpType.add)

    l1_wh(5, 4096.0, LPW5, 2)
    l1_wh(6, 262144.0, LPH5, 3)

    nc.vector.tensor_tensor(out=L1A[:], in0=L1A[:], in1=POSB2[:], op=AluOpType.mult)
    L1Q = cpool.tile([QP, C], F32)
    nc.vector.tensor_reduce(out=L1Q[:], in_=L1A[:].rearrange("p (i c) -> p c i", c=C),
                            axis=AX.X, op=AluOpType.add)
    L1C_p = ppool.tile([1, C], F32, tag="pmm")
    nc.tensor.matmul(out=L1C_p[:], lhsT=ONESC[:], rhs=L1Q[:], start=True, stop=True)
    L1C = cpool.tile([1, C], F32)
    nc.scalar.copy(out=L1C[:], in_=L1C_p[:])

    # ================= outputs =================
    chp = ppool.tile([1, C], F32, tag="pmm")
    nc.tensor.transpose(out=chp[:], in_=CH[:, :1], identity=IDENT[:C, :C])
    CHR = cpool.tile([1, C], F32)
    nc.scalar.copy(out=CHR[:], in_=chp[:])
    nc.sync.dma_start(out=out_part[0:1, :], in_=NPC[:])
    nc.sync.dma_start(out=out_part[1:2, :], in_=CPC[:])
    nc.sync.dma_start(out=out_part[2:3, :], in_=CHR[:])
    nc.sync.dma_start(out=out_part[3:4, :], in_=L1C[:])


# ---------------- host reference partials (for validation) ----------------
def numpy_partials(scores_nc, locs_nc, boxes_nc, priors):
    def cxcy_to_xy(c):
        return np.concatenate([c[..., :2] - c[..., 2:] / 2, c[..., :2] + c[..., 2:] / 2], -1)

    priors_xy = cxcy_to_xy(priors)
    n_pos = np.zeros(C); conf_pos = np.zeros(C); conf_hard = np.zeros(C); l1s = np.zeros(C)
    for c in range(C):
        b = boxes_nc[c]
        lo = np.maximum(b[:, None, :2], priors_xy[None, :, :2])
        hi = np.minimum(b[:, None, 2:], priors_xy[None, :, 2:])
        inter = np.prod(np.clip(hi - lo, 0, None), -1)
        aa = np.prod(b[:, 2:] - b[:, :2], -1)
        ab = np.prod(priors_xy[:, 2:] - priors_xy[:, :2], -1)
        ov = (inter / (aa[:, None] + ab[None, :] - inter)).astype(np.float32)
        ofp = ov.argmax(0); vfp = ov.max(0)
        pfo = ov.argmax(1)
        ofp[pfo] = np.arange(M); vfp[pfo] = 1.0
        pos = vfp >= 0.5
        n_pos[c] = pos.sum()
        d = (scores_nc[c, :, 1] - scores_nc[c, :, 0]).astype(np.float32)
        ce = np.logaddexp(0, np.where(pos, -d, d)).astype(np.float32)
        conf_pos[c] = ce[pos].sum()
        ce_neg = np.where(pos, 0, ce)
        k = int(3 * n_pos[c])
        srt = np.sort(ce_neg)[::-1]
        conf_hard[c] = srt[:k].sum()
        bm = b[ofp]
        bcx = (bm[:, 0] + bm[:, 2]) / 2; bcy = (bm[:, 1] + bm[:, 3]) / 2
        bw = bm[:, 2] - bm[:, 0]; bh = bm[:, 3] - bm[:, 1]
        gcx = (bcx - priors[:, 0]) / (priors[:, 2] / 10)
        gcy = (bcy - priors[:, 1]) / (priors[:, 3] / 10)
        gw = np.log(bw / priors[:, 2]) * 5
        gh = np.log(bh / priors[:, 3]) * 5
        tl = np.stack([gcx, gcy, gw, gh], -1)
        l1 = np.abs(locs_nc[c] - tl).sum(-1) * pos
        l1s[c] = l1.sum()
    return np.stack([n_pos, conf_pos, conf_hard, l1s]).astype(np.float32)


def combine_partials(parts):
    tot = np.sum([p[:4] for p in parts], axis=0).astype(np.float64)
    n_pos_c, conf_pos_c, conf_hard_c, l1_c = tot
    loc_loss_c = l1_c / np.maximum(n_pos_c * 4.0, 1.0)
    safe = np.maximum(n_pos_c, 1.0)
    loss_c = np.where(n_pos_c > 0, (conf_pos_c + conf_hard_c + 1.0 * loc_loss_c) / safe, 0.0) / C
    return np.float32(loss_c.sum())


# ======================= entry point =======================
import os as _os

LAST_EXEC_NS = None
_COMPILED = None
N_CORES = 8


def _install_ntff_hook():
    """Provide antenv.axon_hooks if the image lacks it, so trace=True works."""
    import sys as _sys, types as _types
    try:
        from antenv.axon_hooks import get_axon_ntff_profile_hook  # noqa
        return
    except ImportError:
        pass
    mod = _types.ModuleType("antenv.axon_hooks")
    _h = {"hook": None}
    mod.set_axon_ntff_profile_hook = lambda h: _h.__setitem__("hook", h)
    mod.get_axon_ntff_profile_hook = lambda: _h["hook"]
    _sys.modules["antenv.axon_hooks"] = mod
    try:
        import antenv
        antenv.axon_hooks = mod
        from trn_agent_boot.trn_boot import _ntff_profile_via_ctypes
        mod.set_axon_ntff_profile_hook(_ntff_profile_via_ctypes("/opt/axon/libaxon_pjrt.so"))
    except Exception:
        pass


def _build_module():
    global _COMPILED
    if _COMPILED is not None:
        return _COMPILED
    import concourse.bacc as bacc
    from concourse.bass_interp import get_hw_module

    shapes = {
        "scores_pad": (C, QP * 138),
        "locs_pad": (C, QP * 276),
        "boxes_t": (1, C * M * 4),
        "priors_t": (PP, 4),
        "ident": (QP, QP),
        "ind120": (SEL_ROWS, C),
        "indT": (C, SEL_ROWS),
        "laterm": (QP, 3 * CM),
        "coffs": (QP, 3),
        "mvals": (QP, 3),
        "padmask": (QP, IC),
    }
    nc = bacc.Bacc("TRN2", target_bir_lowering=False, debug=False, enable_asserts=False)
    in_aps = {}
    for name, shp in shapes.items():
        t = nc.dram_tensor(name, shp, mybir.dt.float32, kind="ExternalInput")
        in_aps[name] = t.ap()
    out_t = nc.dram_tensor("part", (8, C), mybir.dt.float32, kind="ExternalOutput")
    out_aps = {"part": out_t.ap()}
    with tile.TileContext(nc, trace_sim=False) as tc:
        build_kernel(tc, out_aps, in_aps)
    nc.compile()
    nc.m = get_hw_module(nc.m)
    _COMPILED = nc
    return nc


def kernel(predicted_locs, predicted_scores, boxes, labels, priors_cxcy):
    """Full (unsharded) inputs -> full scalar output. Data-parallel over N on 8 cores."""
    global LAST_EXEC_NS
    from concourse import bass_utils

    predicted_locs = np.ascontiguousarray(predicted_locs, np.float32)
    predicted_scores = np.ascontiguousarray(predicted_scores, np.float32)
    boxes = np.ascontiguousarray(boxes, np.float32)
    priors_cxcy = np.ascontiguousarray(priors_cxcy, np.float32)

    shared = prep_shared_inputs(priors_cxcy)
    in_maps = []
    for n in range(N_CORES):
        m = dict(shared)
        m.update(prep_core_inputs(predicted_scores[n], predicted_locs[n], boxes[n]))
        in_maps.append(m)

    nc = _build_module()
    trace = _os.environ.get("KERNEL_TRACE", "0") == "1"
    if trace:
        _install_ntff_hook()
    res = bass_utils.run_bass_kernel_spmd(
        nc, in_maps, core_ids=list(range(N_CORES)), trace=trace,
    )
    LAST_EXEC_NS = res.exec_time_ns
    parts = [res.results[n]["part"] for n in range(N_CORES)]
    return combine_partials(parts)


# revision 15
# speedup vs baseline: 1.1469x; 1.1469x over previous
"""MultiBox loss kernel for Trainium2 (Bass/Tile).

Layout: per core, one sample n. Priors padded 8732 -> 8832 = 128*69.
Prior p lives at (partition q = p // 69, column i = p % 69).
Dense tiles are (128, 1380) "i-major": free index i*20 + c.
Box-broadcast tiles are (128, 320) c-major: free index c*16 + m.

Match score: d = ln(inter) - ln(S') with S' = (areaA+areaB)*e^-SIG, so
d = ln(inter/S) + SIG.  iou >= 0.5  <=>  d >= SIG + ln(1/3).
Packing: qm = (d & ~0x7FF) | (15-m); QPA accumulates (qm | 16*(68-i))
max over i.  DVE STT ops (2x_2p mode, 0.5 cyc/el fp32) carry most of
the elementwise work; bitwise ORs of raw-bit codes ride STT scalars
(the STT scalar path preserves arbitrary bit patterns).
"""
import numpy as np

import concourse.bass as bass
import concourse.mybir as mybir
from concourse import tile
from concourse.alu_op_type import AluOpType
from concourse.bass import IndirectOffsetOnAxis

# ---------------- constants ----------------
C, P, M = 20, 8732, 16
QP, I = 128, 69           # partitions x columns
PP = QP * I               # 8832
CM = C * M                # 320
IC = I * C                # 1380
NEG_POS_RATIO = 3.0
SIG = 4.6                 # score shift
ESIG = float(np.exp(-SIG))
_thr = np.float32(np.float32(np.log(np.float32(1.0 / 3.0))) + np.float32(SIG))
THRP = float(np.int32(int(_thr.view(np.int32)) & ~0x7FF).view(np.float32))
SEL_ROWS, SEL_F = 80, 2208   # selection layout: 4 partitions x (69*32) per class
BISECT_ITERS = 10
DUMP_OFF = 10_000_000     # out-of-bounds scatter offset (dropped)
LN_MIN, LN_RANGE = -15.2, 9.3   # range of 5*ln(w) for box sizes

F32 = mybir.dt.float32
I32 = mybir.dt.int32
AF = mybir.ActivationFunctionType
AX = mybir.AxisListType

# ---------------- custom DVE ops ----------------
_REGISTERED = {}


def _register_op(name, spec, subdim=False):
    if name in _REGISTERED:
        return _REGISTERED[name]
    from concourse.dve_ops import DveOp, OPS, CUSTOM_DVE_SPECS, _SUB_OPCODE_FOR_NAME, _CUSTOM_DVE_ROW_BASE
    from concourse.dve_spec import lower, _has_src1
    from concourse.dve_uop import DveOpSpec
    row = _CUSTOM_DVE_ROW_BASE + len(OPS)
    assert row < 0x20
    _SUB_OPCODE_FOR_NAME[name] = row
    shas = {}
    for ver in ("v3", "v4"):
        s = DveOpSpec(name=name, opcode=row, uops=lower(spec, ver=ver), rd1_en=_has_src1(spec))
        shas[ver] = s.sha(ver)
    op = DveOp(name, spec, subdim=subdim, uops_sha=shas)
    OPS.append(op)
    CUSTOM_DVE_SPECS[name] = spec
    _REGISTERED[name] = op
    return op


def get_ops():
    from concourse.dve_spec import (Spec, Src0, Src1, C0, C1, C2, Zero,
                                    maxx, minn, select, AluOp, Idx)

    ovl = _register_op("ANT_OVL", Spec(
        body=maxx(minn(Src0, C0) - maxx(Src1, C1), C2),
        reference=lambda in0, in1, s0, s1, imm2: np.maximum(
            np.minimum(in0, s0) - np.maximum(in1, s1), imm2).astype(np.float32),
    ))

    def _idxmax_ref(in0, in1, s0, s1, imm2):
        n = in0.shape[1]
        out = np.where(in0 >= s0, s1 - np.arange(n)[None, :], 0.0).astype(np.float32)
        return out, out.max(axis=1, keepdims=True)

    idxmax = _register_op("ANT_IDXMAX", Spec(
        body=select(Src0 >= C0, C1 - Idx, Zero),
        accum=AluOp.MAX,
        reference=_idxmax_ref,
    ))

    def _selmax_ref(in0, in1, s0, s1, imm2):
        out = np.where(in0 >= s0, in1, 0.0).astype(np.float32)
        return out, out.max(axis=1, keepdims=True)

    selmax = _register_op("ANT_SELMAX", Spec(
        body=select(Src0 >= C0, Src1, Zero),
        accum=AluOp.MAX,
        reference=_selmax_ref,
    ))

    def _sumgt_ref(in0, in1, s0, s1, imm2):
        out = np.where(in0 > s0, in0, 0.0).astype(np.float32)
        return out, out.sum(axis=1, keepdims=True, dtype=np.float32)

    sumgt = _register_op("ANT_SUMGT", Spec(
        body=select(Src0 > C0, Src0, Zero),
        accum=AluOp.ADD,
        reference=_sumgt_ref,
    ))
    return ovl, idxmax, selmax, sumgt


# ---------------- host-side input prep ----------------
def prep_core_inputs(scores_nc, locs_nc, boxes_nc):
    sc = np.zeros((C, QP * 138), np.float32)
    sc[:, : P * 2] = scores_nc.reshape(C, P * 2)
    lc = np.zeros((C, QP * 276), np.float32)
    lc[:, : P * 4] = locs_nc.reshape(C, P * 4)
    return {
        "scores_pad": sc,
        "locs_pad": lc,
        "boxes_t": boxes_nc.reshape(1, CM * 4).astype(np.float32),
    }


def prep_shared_inputs(priors):
    pr = np.zeros((PP, 4), np.float32)
    pr[:P] = priors
    pr[P:, 0] = 50.0 + np.arange(PP - P)
    pr[P:, 1] = 50.0
    pr[P:, 2] = 0.01
    pr[P:, 3] = 0.01

    ident = np.eye(QP, dtype=np.float32)
    ind120 = np.zeros((SEL_ROWS, C), np.float32)
    for k in range(SEL_ROWS):
        ind120[k, k // 4] = 1.0
    indT = np.ascontiguousarray(ind120.T)
    later = np.zeros((QP, QP), np.float32)
    for a in range(QP):
        for b in range(QP):
            if b > a and b // M == a // M:
                later[a, b] = 1.0
    pidx = np.arange(QP)[:, None] * I + np.arange(I)[None, :]   # (128, 69)
    padmask = (pidx < P).astype(np.float32)[:, :, None].repeat(C, 2).reshape(QP, IC)
    part = np.arange(QP)
    coffs = np.stack([((b * QP + part) // M).astype(np.float32) for b in range(3)], 1)
    mvals = np.stack([(15.0 - (b * QP + part) % M).astype(np.float32) for b in range(3)], 1)
    return {
        "priors_t": pr,
        "ident": ident,
        "ind120": ind120,
        "indT": indT,
        "later": later,
        "coffs": coffs,
        "mvals": mvals,
        "padmask": padmask,
    }


# ---------------- the kernel ----------------
def build_kernel(tc, outs, ins):
    nc = tc.nc
    OVL, IDXMAX, SELMAX, SUMGT = get_ops()

    out_part = outs["part"]      # (8, 20) f32

    from contextlib import ExitStack
    with ExitStack() as ctx:
        cpool = ctx.enter_context(tc.tile_pool(name="const", bufs=1))
        lpool = ctx.enter_context(tc.tile_pool(name="loop", bufs=2))
        ppool = ctx.enter_context(tc.tile_pool(name="psum", bufs=2, space="PSUM"))
        dpool = ctx.enter_context(tc.tile_pool(name="dram", bufs=1, space="DRAM"))
        _build(nc, tc, cpool, lpool, ppool, dpool, ins, out_part,
               OVL, IDXMAX, SELMAX, SUMGT)


def _build(nc, tc, cpool, lpool, ppool, dpool, ins, out_part, OVL, IDXMAX, SELMAX, SUMGT):
    scores = ins["scores_pad"]
    locs = ins["locs_pad"]
    boxes_t = ins["boxes_t"]
    priors_t = ins["priors_t"]
    stt = nc.vector.scalar_tensor_tensor

    # ---- load constants / inputs ----
    PR = cpool.tile([QP, I, 4], F32)
    nc.sync.dma_start(out=PR[:], in_=priors_t.rearrange("(q i) k -> q i k", q=QP))
    IDENT = cpool.tile([QP, QP], F32)
    nc.sync.dma_start(out=IDENT[:], in_=ins["ident"])
    IND120 = cpool.tile([SEL_ROWS, C], F32)
    nc.sync.dma_start(out=IND120[:], in_=ins["ind120"])
    INDT = cpool.tile([C, SEL_ROWS], F32)
    nc.sync.dma_start(out=INDT[:], in_=ins["indT"])
    LATER = cpool.tile([QP, QP], F32)
    nc.sync.dma_start(out=LATER[:], in_=ins["later"])

    SC = cpool.tile([QP, C, 138], F32)
    nc.sync.dma_start(out=SC[:], in_=scores.rearrange("c (q e) -> q c e", q=QP))
    PL = cpool.tile([QP, C, 276], F32)
    nc.sync.dma_start(out=PL[:], in_=locs.rearrange("c (q e) -> q c e", q=QP))
    BT = cpool.tile([1, CM * 4], F32)
    nc.sync.dma_start(out=BT[:], in_=boxes_t)

    PADM = cpool.tile([QP, IC], F32, tag="padm")
    nc.sync.dma_start(out=PADM[:], in_=ins["padmask"])
    CONSTI = cpool.tile([QP, 9], I32)
    # 0: pack mask ~0x7FF, 1: col extract 0x7F0, 2: m extract 0xF,
    # 3: 63, 4: 0xFC0, 5: 0x3F000, 6: 0xFC0000, 7: unused, 8: 0
    for _k, _v in enumerate([~0x7FF, 0x7F0, 0xF, 63, 0xFC0, 0x3F000, 0xFC0000, 0, 0]):
        nc.vector.memset(CONSTI[:, _k:_k + 1], _v)
    # m-codes (15-m) at free index c*16+m
    MDC = cpool.tile([QP, CM], I32)
    nc.gpsimd.iota(MDC[:], pattern=[[0, C], [-1, M]], base=15, channel_multiplier=0)
    # column codes 16*(68-i)
    QPK16 = cpool.tile([QP, I], I32)
    nc.gpsimd.iota(QPK16[:], pattern=[[-16, I]], base=16 * 68, channel_multiplier=0)
    COFF = cpool.tile([QP, 3], F32)
    nc.sync.dma_start(out=COFF[:], in_=ins["coffs"])
    VALS = cpool.tile([QP, 3], F32)
    nc.sync.dma_start(out=VALS[:], in_=ins["mvals"])

    # ---- prior-derived tiles (128, 69) ----
    pcx = PR[:, :, 0]
    pcy = PR[:, :, 1]
    pw = PR[:, :, 2]
    ph = PR[:, :, 3]
    PX1 = cpool.tile([QP, I], F32)
    PX2 = cpool.tile([QP, I], F32)
    PY1 = cpool.tile([QP, I], F32)
    PY2 = cpool.tile([QP, I], F32)
    PAREA = cpool.tile([QP, I], F32)   # prior area pre-scaled by exp(-SIG)
    stt(out=PX1[:], in0=pw, scalar=-0.5, in1=pcx, op0=AluOpType.mult, op1=AluOpType.add)
    stt(out=PX2[:], in0=pw, scalar=0.5, in1=pcx, op0=AluOpType.mult, op1=AluOpType.add)
    stt(out=PY1[:], in0=ph, scalar=-0.5, in1=pcy, op0=AluOpType.mult, op1=AluOpType.add)
    stt(out=PY2[:], in0=ph, scalar=0.5, in1=pcy, op0=AluOpType.mult, op1=AluOpType.add)
    stt(out=PAREA[:], in0=pw, scalar=ESIG, in1=ph, op0=AluOpType.mult, op1=AluOpType.mult)
    IPW = cpool.tile([QP, I], F32)   # 10 / pw
    IPH = cpool.tile([QP, I], F32)
    scr69 = cpool.tile([QP, I], F32)
    nc.vector.reciprocal_approx_accurate(out=IPW[:], in_=pw, scratch=scr69[:])
    nc.vector.reciprocal_approx_accurate(out=IPH[:], in_=ph, scratch=scr69[:])
    nc.vector.tensor_scalar_mul(IPW[:], IPW[:], 10.0)
    nc.vector.tensor_scalar_mul(IPH[:], IPH[:], 10.0)
    LPW5 = cpool.tile([QP, I], F32)  # 5*ln(pw)
    LPH5 = cpool.tile([QP, I], F32)
    nc.scalar.activation(out=LPW5[:], in_=pw, func=AF.Ln)
    nc.scalar.activation(out=LPH5[:], in_=ph, func=AF.Ln)
    nc.vector.tensor_scalar_mul(LPW5[:], LPW5[:], 5.0)
    nc.vector.tensor_scalar_mul(LPH5[:], LPH5[:], 5.0)

    # per-prior l1 helper tiles (no loop deps; emitted early for overlap)
    IPW63 = cpool.tile([QP, I], F32)
    IPH63 = cpool.tile([QP, I], F32)
    nc.vector.tensor_scalar_mul(IPW63[:], IPW[:], 1.0 / 63.0)
    nc.vector.tensor_scalar_mul(IPH63[:], IPH[:], 1.0 / (63.0 * 64.0))
    PCXI = cpool.tile([QP, I], F32)
    PCYI = cpool.tile([QP, I], F32)
    nc.vector.tensor_tensor(out=PCXI[:], in0=pcx, in1=IPW[:], op=AluOpType.mult)
    nc.vector.tensor_tensor(out=PCYI[:], in0=pcy, in1=IPH[:], op=AluOpType.mult)

    # ---- box-derived broadcast tiles (128, 320) c-major ----
    bx1v = BT[:, 0::4]
    by1v = BT[:, 1::4]
    bx2v = BT[:, 2::4]
    by2v = BT[:, 3::4]
    BD = cpool.tile([1, CM * 10], F32, tag="cbslot")
    s = [BD[:, k * CM:(k + 1) * CM] for k in range(10)]
    # 0 bx1, 1 bx2, 2 by1, 3 by2, 4 bcx, 5 bcy, 6 areab*e^-SIG, 7 lnbw5, 8 lnbh5, 9 quad
    nc.vector.tensor_copy(out=s[0], in_=bx1v)
    nc.vector.tensor_copy(out=s[1], in_=bx2v)
    nc.vector.tensor_copy(out=s[2], in_=by1v)
    nc.vector.tensor_copy(out=s[3], in_=by2v)
    t1 = cpool.tile([1, CM], F32)
    nc.vector.tensor_tensor(out=t1[:], in0=bx1v, in1=bx2v, op=AluOpType.add)
    nc.vector.tensor_scalar_mul(s[4], t1[:], 0.5)
    nc.vector.tensor_tensor(out=t1[:], in0=by1v, in1=by2v, op=AluOpType.add)
    nc.vector.tensor_scalar_mul(s[5], t1[:], 0.5)
    tbw = cpool.tile([1, CM], F32)
    tbh = cpool.tile([1, CM], F32)
    nc.vector.tensor_tensor(out=tbw[:], in0=bx2v, in1=bx1v, op=AluOpType.subtract)
    nc.vector.tensor_tensor(out=tbh[:], in0=by2v, in1=by1v, op=AluOpType.subtract)
    stt(out=s[6], in0=tbw[:], scalar=ESIG, in1=tbh[:], op0=AluOpType.mult, op1=AluOpType.mult)
    nc.scalar.activation(out=s[7], in_=tbw[:], func=AF.Ln)
    nc.scalar.activation(out=s[8], in_=tbh[:], func=AF.Ln)
    nc.vector.tensor_scalar_mul(s[7], s[7], 5.0)
    nc.vector.tensor_scalar_mul(s[8], s[8], 5.0)
    # quad code: round(bcx*63) + 64*round(bcy*63) + 64^2*round((lnw5-LN_MIN)*63/LN_RANGE) + 64^3*(...)
    enc = cpool.tile([1, CM], I32)
    encf = cpool.tile([1, CM], F32)
    quad = s[9]
    nc.vector.tensor_scalar(out=enc[:], in0=s[4], scalar1=63.0, scalar2=0.5,
                            op0=AluOpType.mult, op1=AluOpType.add)
    nc.vector.tensor_copy(out=quad, in_=enc[:])
    nc.vector.tensor_scalar(out=enc[:], in0=s[5], scalar1=63.0, scalar2=0.5,
                            op0=AluOpType.mult, op1=AluOpType.add)
    nc.vector.tensor_copy(out=encf[:], in_=enc[:])
    stt(out=quad, in0=encf[:], scalar=64.0, in1=quad, op0=AluOpType.mult, op1=AluOpType.add)
    nc.vector.tensor_scalar(out=enc[:], in0=s[7], scalar1=63.0 / LN_RANGE,
                            scalar2=-LN_MIN * 63.0 / LN_RANGE + 0.5,
                            op0=AluOpType.mult, op1=AluOpType.add)
    nc.vector.tensor_copy(out=encf[:], in_=enc[:])
    stt(out=quad, in0=encf[:], scalar=4096.0, in1=quad, op0=AluOpType.mult, op1=AluOpType.add)
    nc.vector.tensor_scalar(out=enc[:], in0=s[8], scalar1=63.0 / LN_RANGE,
                            scalar2=-LN_MIN * 63.0 / LN_RANGE + 0.5,
                            op0=AluOpType.mult, op1=AluOpType.add)
    nc.vector.tensor_copy(out=encf[:], in_=enc[:])
    stt(out=quad, in0=encf[:], scalar=262144.0, in1=quad, op0=AluOpType.mult, op1=AluOpType.add)

    ONES1 = cpool.tile([1, QP], F32)
    nc.vector.memset(ONES1[:], 1.0)
    BB = cpool.tile([QP, CM * 10], F32)
    tot = CM * 10
    off = 0
    while off < tot:
        w = min(512, tot - off)
        pt = ppool.tile([QP, w], F32, tag="bcast")
        nc.tensor.matmul(out=pt[:], lhsT=ONES1[:], rhs=BD[:, off:off + w], start=True, stop=True)
        nc.scalar.copy(out=BB[:, off:off + w], in_=pt[:])
        off += w
    BX1 = BB[:, 0 * CM:1 * CM]
    BX2 = BB[:, 1 * CM:2 * CM]
    BY1 = BB[:, 2 * CM:3 * CM]
    BY2 = BB[:, 3 * CM:4 * CM]
    BAR = BB[:, 6 * CM:7 * CM]
    QUADB = BB[:, 9 * CM:10 * CM]

    # ---- CE (no dependency on matching; emitted early for engine overlap) ----
    DM = cpool.tile([QP, IC], F32, tag="dm")
    sc4 = SC[:].rearrange("p c (i two) -> p c i two", two=2)
    dm3 = DM[:].rearrange("p (i c) -> p i c", c=C)
    stt(out=dm3.rearrange("p i c -> p c i"), in0=sc4[:, :, :, 1], scalar=1.0,
        in1=sc4[:, :, :, 0], op0=AluOpType.mult, op1=AluOpType.subtract)
    CE = cpool.tile([QP, IC], F32)
    nc.scalar.activation(out=CE[:], in_=DM[:], func=AF.Exp)
    nc.scalar.activation(out=CE[:], in_=CE[:], func=AF.Ln, bias=1.0)

    # init FMD scratch early (no deps)
    FMD = dpool.tile([PP * C, 1], F32)
    NEG1 = cpool.tile([QP, IC], F32, tag="l1a")
    nc.vector.memset(NEG1[:], -1.0)
    nc.sync.dma_start(out=FMD[:].rearrange("(q f) one -> q (f one)", q=QP), in_=NEG1[:])

    # ---- accumulators ----
    QMM = cpool.tile([QP, I, C], F32)
    QPA = cpool.tile([QP, CM], F32)
    nc.vector.memset(QPA[:], 0.0)
    MDCF = MDC[:].bitcast(F32)

    # ================= main loop over columns i =================
    for i in range(I):
        xov = lpool.tile([QP, CM], F32, tag="xov")
        nc.vector._custom_dve(OVL, out=xov[:], in0=BX2, in1=BX1,
                              s0=PX2[:, i:i + 1], s1=PX1[:, i:i + 1], imm2=1e-18)
        yov = lpool.tile([QP, CM], F32, tag="yov")
        nc.vector._custom_dve(OVL, out=yov[:], in0=BY2, in1=BY1,
                              s0=PY2[:, i:i + 1], s1=PY1[:, i:i + 1], imm2=1e-18)
        inter = lpool.tile([QP, CM], F32, tag="inter")
        stt(out=inter[:], in0=xov[:], scalar=1.0, in1=yov[:],
            op0=AluOpType.mult, op1=AluOpType.mult)
        S = lpool.tile([QP, CM], F32, tag="S")
        nc.scalar.activation(out=S[:], in_=BAR, func=AF.Identity,
                             bias=PAREA[:, i:i + 1], scale=1.0)
        lnI = lpool.tile([QP, CM], F32, tag="lnI")
        nc.scalar.activation(out=lnI[:], in_=inter[:], func=AF.Ln)
        lnS = lpool.tile([QP, CM], F32, tag="lnS")
        nc.scalar.activation(out=lnS[:], in_=S[:], func=AF.Ln)
        d = lpool.tile([QP, CM], F32, tag="d")
        stt(out=d[:], in0=lnI[:], scalar=1.0, in1=lnS[:],
            op0=AluOpType.mult, op1=AluOpType.subtract)
        qm = lpool.tile([QP, CM], F32, tag="qm")
        stt(out=qm[:].bitcast(I32), in0=d[:].bitcast(I32), scalar=CONSTI[:, 0:1],
            in1=MDC[:], op0=AluOpType.bitwise_and, op1=AluOpType.bitwise_or)
        qp = lpool.tile([QP, CM], F32, tag="qp")
        stt(out=qp[:].bitcast(I32), in0=qm[:].bitcast(I32), scalar=QPK16[:, i:i + 1],
            in1=MDC[:], op0=AluOpType.bitwise_or, op1=AluOpType.bitwise_or)
        stt(out=QPA[:], in0=qp[:], scalar=1.0, in1=QPA[:],
            op0=AluOpType.mult, op1=AluOpType.max)
        nc.vector.tensor_reduce(out=QMM[:, i, :],
                                in_=qm[:].rearrange("p (c m) -> p c m", m=M),
                                axis=AX.X, op=AluOpType.max)

    QMMf = QMM[:].rearrange("p i c -> p (i c)")
    QMMi = QMMf.bitcast(I32)

    # ================= pos mask, m* =================
    POSB = cpool.tile([QP, IC], F32, tag="posb")
    nc.vector.tensor_scalar(out=POSB[:], in0=QMMf, scalar1=THRP, scalar2=0.0,
                            op0=AluOpType.is_ge, op1=AluOpType.max)
    # m-code (15-m) in low 4 bits
    MSI = cpool.tile([QP, IC], I32, tag="ic_int")
    stt(out=MSI[:], in0=QMMi, scalar=CONSTI[:, 2:3],
        in1=CONSTI[:, 8:9].to_broadcast([QP, IC]),
        op0=AluOpType.bitwise_and, op1=AluOpType.bitwise_or)
    MS = cpool.tile([QP, IC], F32)
    nc.vector.tensor_copy(out=MS[:], in_=MSI[:])

    # ================= prior_for_obj (forced positives) =================
    QPAf = QPA[:]
    PSTARI = cpool.tile([QP, 3], I32)
    for b in range(3):
        w = 128 if b < 2 else 64
        tp = ppool.tile([QP, QP], F32, tag="ptr")
        nc.tensor.transpose(out=tp[:w, :], in_=QPAf[:, b * QP:b * QP + w], identity=IDENT[:])
        TQ = lpool.tile([QP, QP], F32, tag="TQ")
        nc.scalar.copy(out=TQ[:w, :], in_=tp[:w, :])
        vmax = lpool.tile([QP, 1], F32, tag="vmax")
        nc.vector.tensor_reduce(out=vmax[:w], in_=TQ[:w, :], axis=AX.X, op=AluOpType.max)
        qd = lpool.tile([QP, 1], F32, tag="qd")
        sc1 = lpool.tile([QP, QP], F32, tag="sc1")
        nc.vector._custom_dve(IDXMAX, out=sc1[:w, :], accum_out=qd[:w], in0=TQ[:w, :],
                              s0=vmax[:w], s1=127.0)
        TLI = lpool.tile([QP, QP], I32, tag="TLI")
        stt(out=TLI[:w, :], in0=TQ[:w, :].bitcast(I32), scalar=CONSTI[:w, 1:2],
            in1=CONSTI[:w, 8:9].to_broadcast([w, QP]),
            op0=AluOpType.bitwise_and, op1=AluOpType.bitwise_or)
        TLF = lpool.tile([QP, QP], F32, tag="TLF")
        nc.vector.tensor_copy(out=TLF[:w, :], in_=TLI[:w, :])
        colv = lpool.tile([QP, 1], F32, tag="ilow")
        sc2 = lpool.tile([QP, QP], F32, tag="sc2")
        nc.vector._custom_dve(SELMAX, out=sc2[:w, :], accum_out=colv[:w], in0=TQ[:w, :],
                              in1=TLF[:w, :], s0=vmax[:w])
        # p* = (127 - qd)*69 + (68 - colv/16)
        pst = lpool.tile([QP, 1], F32, tag="pst")
        nc.vector.tensor_scalar(out=pst[:w], in0=qd[:w], scalar1=-69.0,
                                scalar2=float(127 * 69 + 68),
                                op0=AluOpType.mult, op1=AluOpType.add)
        stt(out=pst[:w], in0=colv[:w], scalar=-1.0 / 16.0, in1=pst[:w],
            op0=AluOpType.mult, op1=AluOpType.add)
        # dedup: later m with same p* in same class wins
        tpp = ppool.tile([QP, QP], F32, tag="ptr")
        nc.tensor.transpose(out=tpp[:, :w], in_=pst[:w, :1].to_broadcast([w, QP]),
                            identity=IDENT[:w, :w])
        PTT = lpool.tile([QP, QP], F32, tag="PTT")
        nc.scalar.copy(out=PTT[:, :w], in_=tpp[:, :w])
        EQM = lpool.tile([QP, QP], F32, tag="EQM")
        nc.vector.tensor_tensor(out=EQM[:w, :w], in0=pst[:w, :1].to_broadcast([w, w]),
                                in1=PTT[:w, :w], op=AluOpType.is_equal)
        nc.vector.tensor_tensor(out=EQM[:w, :w], in0=EQM[:w, :w], in1=LATER[:w, :w],
                                op=AluOpType.mult)
        dom = lpool.tile([QP, 1], F32, tag="dom")
        nc.vector.tensor_reduce(out=dom[:w], in_=EQM[:w, :w], axis=AX.X, op=AluOpType.max)
        # offset = p* * 20 + c; dominated -> +DUMP_OFF (dropped by bounds check)
        offf = lpool.tile([QP, 1], F32, tag="offf")
        stt(out=offf[:w], in0=pst[:w], scalar=20.0, in1=COFF[:w, b:b + 1],
            op0=AluOpType.mult, op1=AluOpType.add)
        stt(out=offf[:w], in0=dom[:w], scalar=float(DUMP_OFF), in1=offf[:w],
            op0=AluOpType.mult, op1=AluOpType.add)
        nc.vector.tensor_copy(out=PSTARI[:w, b:b + 1], in_=offf[:w])

    for b in range(3):
        w = 128 if b < 2 else 64
        nc.gpsimd.indirect_dma_start(
            out=FMD[:],
            out_offset=IndirectOffsetOnAxis(ap=PSTARI[:w, b:b + 1], axis=0),
            in_=VALS[:w, b:b + 1],
            in_offset=None,
            bounds_check=PP * C - 1,
            oob_is_err=False,
        )
    FM = cpool.tile([QP, IC], F32, tag="fm")
    nc.sync.dma_start(out=FM[:], in_=FMD[:].rearrange("(q f) one -> q (f one)", q=QP))

    FGE = cpool.tile([QP, IC], F32)
    nc.vector.tensor_scalar(out=FGE[:], in0=FM[:], scalar1=0.0, scalar2=0.0,
                            op0=AluOpType.is_ge, op1=AluOpType.max)
    POSB2 = POSB
    nc.vector.tensor_tensor(out=POSB2[:], in0=POSB[:], in1=FGE[:], op=AluOpType.max)
    FGEI = cpool.tile([QP, IC], I32, tag="ic_int")
    nc.vector.tensor_copy(out=FGEI[:], in_=FGE[:])
    MS2 = MS
    nc.vector.copy_predicated(out=MS2[:], mask=FGEI[:], data=FM[:])

    # ================= CE pos/neg splits =================
    CEP = cpool.tile([QP, IC], F32, tag="cep")
    stt(out=CEP[:], in0=PADM[:], scalar=1.0, in1=POSB2[:],
        op0=AluOpType.mult, op1=AluOpType.subtract)
    CEN = cpool.tile([QP, C, I], F32, tag="scbslot")
    cen_im = CEN[:].rearrange("p c i -> p i c")
    stt(out=cen_im, in0=CE[:].rearrange("p (i c) -> p i c", c=C), scalar=1.0,
        in1=CEP[:].rearrange("p (i c) -> p i c", c=C),
        op0=AluOpType.mult, op1=AluOpType.mult)
    CPT = cpool.tile([QP, IC], F32, tag="gt")
    stt(out=CPT[:], in0=CE[:], scalar=1.0, in1=DM[:],
        op0=AluOpType.mult, op1=AluOpType.subtract)
    stt(out=CPT[:], in0=CPT[:], scalar=1.0, in1=POSB2[:],
        op0=AluOpType.mult, op1=AluOpType.mult)

    # ================= counts / class sums =================
    NPQ = cpool.tile([QP, C], F32)
    nc.vector.tensor_reduce(out=NPQ[:], in_=POSB2[:].rearrange("p (i c) -> p c i", c=C),
                            axis=AX.X, op=AluOpType.add)
    CPQ = cpool.tile([QP, C], F32)
    nc.vector.tensor_reduce(out=CPQ[:], in_=CPT[:].rearrange("p (i c) -> p c i", c=C),
                            axis=AX.X, op=AluOpType.add)
    ONESC = cpool.tile([QP, 1], F32)
    nc.vector.memset(ONESC[:], 1.0)
    NPC_p = ppool.tile([1, C], F32, tag="pmm")
    nc.tensor.matmul(out=NPC_p[:], lhsT=ONESC[:], rhs=NPQ[:], start=True, stop=True)
    CPC_p = ppool.tile([1, C], F32, tag="pmm")
    nc.tensor.matmul(out=CPC_p[:], lhsT=ONESC[:], rhs=CPQ[:], start=True, stop=True)
    NPC = cpool.tile([1, C], F32)
    nc.scalar.copy(out=NPC[:], in_=NPC_p[:])
    CPC = cpool.tile([1, C], F32)
    nc.scalar.copy(out=CPC[:], in_=CPC_p[:])

    kp = ppool.tile([C, 1], F32, tag="pmm")
    nc.tensor.transpose(out=kp[:], in_=NPC[:], identity=IDENT[:1, :1])
    KC = cpool.tile([C, 1], F32)
    nc.scalar.copy(out=KC[:], in_=kp[:])
    nc.vector.tensor_scalar_mul(KC[:], KC[:], NEG_POS_RATIO)

    # ================= hard-negative selection =================
    CB = cpool.tile([SEL_ROWS, SEL_F], F32, tag="cbslot")
    for c in range(C):
        nc.sync.dma_start(out=CB[c * 4:(c + 1) * 4, :], in_=CEN[:, c, :])

    LO = cpool.tile([C, 1], F32)
    HI = cpool.tile([C, 1], F32)
    TC_ = cpool.tile([C, 1], F32)
    nc.vector.memset(LO[:], 0.8)
    nc.vector.memset(HI[:], 4.0)
    T120 = cpool.tile([SEL_ROWS, 1], F32)
    CNT6 = cpool.tile([SEL_ROWS, 1], F32)
    CNTC = cpool.tile([C, 1], F32)
    scb = cpool.tile([SEL_ROWS, SEL_F], F32, tag="scbslot")
    for it in range(BISECT_ITERS):
        nc.vector.tensor_tensor(out=TC_[:], in0=LO[:], in1=HI[:], op=AluOpType.add)
        nc.vector.tensor_scalar_mul(TC_[:], TC_[:], 0.5)
        tp120 = ppool.tile([SEL_ROWS, 1], F32, tag="pmm")
        nc.tensor.matmul(out=tp120[:], lhsT=INDT[:], rhs=TC_[:], start=True, stop=True)
        nc.scalar.copy(out=T120[:], in_=tp120[:])
        nc.vector.tensor_scalar(out=scb[:], in0=CB[:], scalar1=T120[:, :1], scalar2=0.0,
                                op0=AluOpType.is_gt, op1=AluOpType.add, accum_out=CNT6[:])
        tpc = ppool.tile([C, 1], F32, tag="pmm")
        nc.tensor.matmul(out=tpc[:], lhsT=IND120[:], rhs=CNT6[:], start=True, stop=True)
        nc.scalar.copy(out=CNTC[:], in_=tpc[:])
        gm = lpool.tile([C, 1], I32, tag="gm")
        nc.vector.tensor_tensor(out=gm[:], in0=CNTC[:], in1=KC[:], op=AluOpType.is_ge)
        nc.vector.copy_predicated(out=LO[:], mask=gm[:], data=TC_[:])
        lm = lpool.tile([C, 1], I32, tag="lm")
        nc.vector.tensor_tensor(out=lm[:], in0=CNTC[:], in1=KC[:], op=AluOpType.is_lt)
        nc.vector.copy_predicated(out=HI[:], mask=lm[:], data=TC_[:])
    tp120 = ppool.tile([SEL_ROWS, 1], F32, tag="pmm")
    nc.tensor.matmul(out=tp120[:], lhsT=INDT[:], rhs=LO[:], start=True, stop=True)
    nc.scalar.copy(out=T120[:], in_=tp120[:])
    SUM6 = cpool.tile([SEL_ROWS, 1], F32)
    nc.vector._custom_dve(SUMGT, out=scb[:], accum_out=SUM6[:], in0=CB[:], s0=T120[:, :1])
    nc.vector.tensor_scalar(out=scb[:], in0=CB[:], scalar1=T120[:, :1], scalar2=0.0,
                            op0=AluOpType.is_gt, op1=AluOpType.add, accum_out=CNT6[:])
    SUMC_p = ppool.tile([C, 1], F32, tag="pmm")
    nc.tensor.matmul(out=SUMC_p[:], lhsT=IND120[:], rhs=SUM6[:], start=True, stop=True)
    CNTC_p = ppool.tile([C, 1], F32, tag="pmm")
    nc.tensor.matmul(out=CNTC_p[:], lhsT=IND120[:], rhs=CNT6[:], start=True, stop=True)
    CH = cpool.tile([C, 1], F32)
    nc.scalar.copy(out=CNTC[:], in_=CNTC_p[:])
    nc.vector.tensor_tensor(out=CH[:], in0=KC[:], in1=CNTC[:], op=AluOpType.subtract)
    nc.vector.tensor_tensor(out=CH[:], in0=CH[:], in1=LO[:], op=AluOpType.mult)
    SUMC = cpool.tile([C, 1], F32)
    nc.scalar.copy(out=SUMC[:], in_=SUMC_p[:])
    nc.vector.tensor_tensor(out=CH[:], in0=CH[:], in1=SUMC[:], op=AluOpType.add)

    # ================= localization loss =================
    # 16-way select of quad-encoded box quantities by m* code (15-m);
    # selects are disjoint so plain adds accumulate G.
    G = cpool.tile([QP, IC], F32, tag="gt")
    g3 = G[:].rearrange("p (i c) -> p i c", c=C)

    def quadview(m):
        return QUADB[:, m::M].unsqueeze(1).to_broadcast([QP, I, C])

    nc.vector.memset(G[:], 0.0)
    ms3 = MS2[:].rearrange("p (i c) -> p i c", c=C)
    TQM = cpool.tile([QP, I, C], F32, tag="tqm")
    tq3 = TQM[:]
    for m in range(M):
        stt(out=tq3, in0=ms3, scalar=float(15 - m), in1=quadview(m),
            op0=AluOpType.is_equal, op1=AluOpType.mult)
        stt(out=g3, in0=tq3, scalar=1.0, in1=g3, op0=AluOpType.mult, op1=AluOpType.add)
    GI = cpool.tile([QP, IC], I32, tag="ic_int")
    nc.vector.tensor_copy(out=GI[:], in_=G[:])

    L1A = cpool.tile([QP, IC], F32, tag="l1a")
    nc.vector.memset(L1A[:], 0.0)
    EC = cpool.tile([QP, IC], F32, tag="dm")
    ECI = cpool.tile([QP, IC], I32, tag="ec_int")
    TM2 = cpool.tile([QP, IC], F32, tag="cep")
    tm3 = TM2[:].rearrange("p (i c) -> p i c", c=C)
    ec3 = EC[:].rearrange("p (i c) -> p i c", c=C)
    pl5 = PL[:].rearrange("p c (i four) -> p c i four", four=4)

    def bc69(t):
        return t[:].unsqueeze(2).to_broadcast([QP, I, C])

    def l1_xy(mask_col, scale_t, pci_t, k_coord):
        stt(out=ECI[:], in0=GI[:], scalar=CONSTI[:, mask_col:mask_col + 1],
            in1=CONSTI[:, 8:9].to_broadcast([QP, IC]),
            op0=AluOpType.bitwise_and, op1=AluOpType.bitwise_or)
        nc.vector.tensor_copy(out=EC[:], in_=ECI[:])
        # A = pl + pcx*ipw ; t = e * (ipw/63/shift); diff = A - t
        plv = pl5[:, :, :, k_coord].rearrange("p c i -> p i c")
        stt(out=tm3, in0=plv, scalar=1.0, in1=bc69(pci_t),
            op0=AluOpType.mult, op1=AluOpType.add)
        stt(out=ec3, in0=ec3, scalar=1.0, in1=bc69(scale_t),
            op0=AluOpType.mult, op1=AluOpType.mult)
        stt(out=tm3, in0=tm3, scalar=1.0, in1=ec3,
            op0=AluOpType.mult, op1=AluOpType.subtract)
        stt(out=TM2[:], in0=TM2[:], scalar=-1.0, in1=TM2[:],
            op0=AluOpType.mult, op1=AluOpType.max)
        stt(out=L1A[:], in0=TM2[:], scalar=1.0, in1=L1A[:],
            op0=AluOpType.mult, op1=AluOpType.add)

    l1_xy(3, IPW63, PCXI, 0)          # cx: e in [0,63], value e/63 * ipw
    l1_xy(4, IPH63, PCYI, 1)          # cy: e-bits at <<6; scale = iph/(63*64)

    # w/h coords: A = pl + lnpw5 - LN_MIN ; t = e * (LN_RANGE/63/shift)
    def l1_wh(mask_col, shift, lp5, k_coord):
        stt(out=ECI[:], in0=GI[:], scalar=CONSTI[:, mask_col:mask_col + 1],
            in1=CONSTI[:, 8:9].to_broadcast([QP, IC]),
            op0=AluOpType.bitwise_and, op1=AluOpType.bitwise_or)
        nc.vector.tensor_copy(out=EC[:], in_=ECI[:])
        plv = pl5[:, :, :, k_coord].rearrange("p c i -> p i c")
        stt(out=tm3, in0=plv, scalar=-LN_MIN, in1=bc69(lp5),
            op0=AluOpType.add, op1=AluOpType.add)
        nc.vector.tensor_scalar_mul(EC[:], EC[:], LN_RANGE / 63.0 / shift)
        stt(out=tm3, in0=tm3, scalar=1.0, in1=ec3,
            op0=AluOpType.mult, op1=AluOpType.subtract)
        stt(out=TM2[:], in0=TM2[:], scalar=-1.0, in1=TM2[:],
            op0=AluOpType.mult, op1=AluOpType.max)
        stt(out=L1A[:], in0=TM2[:], scalar=1.0, in1=L1A[:],
            op0=AluOpType.mult, op1=AluOpType.add)

    l1_wh(5, 4096.0, LPW5, 2)
    l1_wh(6, 262144.0, LPH5, 3)

    stt(out=L1A[:], in0=L1A[:], scalar=1.0, in1=POSB2[:],
        op0=AluOpType.mult, op1=AluOpType.mult)
    L1Q = cpool.tile([QP, C], F32)
    nc.vector.tensor_reduce(out=L1Q[:], in_=L1A[:].rearrange("p (i c) -> p c i", c=C),
                            axis=AX.X, op=AluOpType.add)
    L1C_p = ppool.tile([1, C], F32, tag="pmm")
    nc.tensor.matmul(out=L1C_p[:], lhsT=ONESC[:], rhs=L1Q[:], start=True, stop=True)
    L1C = cpool.tile([1, C], F32)
    nc.scalar.copy(out=L1C[:], in_=L1C_p[:])

    # ================= outputs =================
    chp = ppool.tile([1, C], F32, tag="pmm")
    nc.tensor.transpose(out=chp[:], in_=CH[:, :1], identity=IDENT[:C, :C])
    CHR = cpool.tile([1, C], F32)
    nc.scalar.copy(out=CHR[:], in_=chp[:])
    nc.sync.dma_start(out=out_part[0:1, :], in_=NPC[:])
    nc.sync.dma_start(out=out_part[1:2, :], in_=CPC[:])
    nc.sync.dma_start(out=out_part[2:3, :], in_=CHR[:])
    nc.sync.dma_start(out=out_part[3:4, :], in_=L1C[:])


# ---------------- host reference partials (for validation) ----------------
def numpy_partials(scores_nc, locs_nc, boxes_nc, priors):
    def cxcy_to_xy(c):
        return np.concatenate([c[..., :2] - c[..., 2:] / 2, c[..., :2] + c[..., 2:] / 2], -1)

    priors_xy = cxcy_to_xy(priors)
    n_pos = np.zeros(C); conf_pos = np.zeros(C); conf_hard = np.zeros(C); l1s = np.zeros(C)
    for c in range(C):
        b = boxes_nc[c]
        lo = np.maximum(b[:, None, :2], priors_xy[None, :, :2])
        hi = np.minimum(b[:, None, 2:], priors_xy[None, :, 2:])
        inter = np.prod(np.clip(hi - lo, 0, None), -1)
        aa = np.prod(b[:, 2:] - b[:, :2], -1)
        ab = np.prod(priors_xy[:, 2:] - priors_xy[:, :2], -1)
        ov = (inter / (aa[:, None] + ab[None, :] - inter)).astype(np.float32)
        ofp = ov.argmax(0); vfp = ov.max(0)
        pfo = ov.argmax(1)
        ofp[pfo] = np.arange(M); vfp[pfo] = 1.0
        pos = vfp >= 0.5
        n_pos[c] = pos.sum()
        d = (scores_nc[c, :, 1] - scores_nc[c, :, 0]).astype(np.float32)
        ce = np.logaddexp(0, np.where(pos, -d, d)).astype(np.float32)
        conf_pos[c] = ce[pos].sum()
        ce_neg = np.where(pos, 0, ce)
        k = int(3 * n_pos[c])
        srt = np.sort(ce_neg)[::-1]
        conf_hard[c] = srt[:k].sum()
        bm = b[ofp]
        bcx = (bm[:, 0] + bm[:, 2]) / 2; bcy = (bm[:, 1] + bm[:, 3]) / 2
        bw = bm[:, 2] - bm[:, 0]; bh = bm[:, 3] - bm[:, 1]
        gcx = (bcx - priors[:, 0]) / (priors[:, 2] / 10)
        gcy = (bcy - priors[:, 1]) / (priors[:, 3] / 10)
        gw = np.log(bw / priors[:, 2]) * 5
        gh = np.log(bh / priors[:, 3]) * 5
        tl = np.stack([gcx, gcy, gw, gh], -1)
        l1 = np.abs(locs_nc[c] - tl).sum(-1) * pos
        l1s[c] = l1.sum()
    return np.stack([n_pos, conf_pos, conf_hard, l1s]).astype(np.float32)


def combine_partials(parts):
    tot = np.sum([p[:4] for p in parts], axis=0).astype(np.float64)
    n_pos_c, conf_pos_c, conf_hard_c, l1_c = tot
    loc_loss_c = l1_c / np.maximum(n_pos_c * 4.0, 1.0)
    safe = np.maximum(n_pos_c, 1.0)
    loss_c = np.where(n_pos_c > 0, (conf_pos_c + conf_hard_c + 1.0 * loc_loss_c) / safe, 0.0) / C
    return np.float32(loss_c.sum())


# ======================= entry point =======================
import os as _os

LAST_EXEC_NS = None
_COMPILED = None
N_CORES = 8


def _install_ntff_hook():
    """Provide antenv.axon_hooks if the image lacks it, so trace=True works."""
    import sys as _sys, types as _types
    try:
        from antenv.axon_hooks import get_axon_ntff_profile_hook  # noqa
        return
    except ImportError:
        pass
    mod = _types.ModuleType("antenv.axon_hooks")
    _h = {"hook": None}
    mod.set_axon_ntff_profile_hook = lambda h: _h.__setitem__("hook", h)
    mod.get_axon_ntff_profile_hook = lambda: _h["hook"]
    _sys.modules["antenv.axon_hooks"] = mod
    try:
        import antenv
        antenv.axon_hooks = mod
        from trn_agent_boot.trn_boot import _ntff_profile_via_ctypes
        mod.set_axon_ntff_profile_hook(_ntff_profile_via_ctypes("/opt/axon/libaxon_pjrt.so"))
    except Exception:
        pass


def _build_module():
    global _COMPILED
    if _COMPILED is not None:
        return _COMPILED
    import concourse.bacc as bacc
    from concourse.bass_interp import get_hw_module

    shapes = {
        "scores_pad": (C, QP * 138),
        "locs_pad": (C, QP * 276),
        "boxes_t": (1, C * M * 4),
        "priors_t": (PP, 4),
        "ident": (QP, QP),
        "ind120": (SEL_ROWS, C),
        "indT": (C, SEL_ROWS),
        "later": (QP, QP),
        "coffs": (QP, 3),
        "mvals": (QP, 3),
        "padmask": (QP, IC),
    }
    nc = bacc.Bacc("TRN2", target_bir_lowering=False, debug=False, enable_asserts=False)
    in_aps = {}
    for name, shp in shapes.items():
        t = nc.dram_tensor(name, shp, mybir.dt.float32, kind="ExternalInput")
        in_aps[name] = t.ap()
    out_t = nc.dram_tensor("part", (8, C), mybir.dt.float32, kind="ExternalOutput")
    out_aps = {"part": out_t.ap()}
    with tile.TileContext(nc, trace_sim=False) as tc:
        build_kernel(tc, out_aps, in_aps)
    nc.compile()
    nc.m = get_hw_module(nc.m)
    _COMPILED = nc
    return nc


def kernel(predicted_locs, predicted_scores, boxes, labels, priors_cxcy):
    """Full (unsharded) inputs -> full scalar output. Data-parallel over N on 8 cores."""
    global LAST_EXEC_NS
    from concourse import bass_utils

    predicted_locs = np.ascontiguousarray(predicted_locs, np.float32)
    predicted_scores = np.ascontiguousarray(predicted_scores, np.float32)
    boxes = np.ascontiguousarray(boxes, np.float32)
    priors_cxcy = np.ascontiguousarray(priors_cxcy, np.float32)

    shared = prep_shared_inputs(priors_cxcy)
    in_maps = []
    for n in range(N_CORES):
        m = dict(shared)
        m.update(prep_core_inputs(predicted_scores[n], predicted_locs[n], boxes[n]))
        in_maps.append(m)

    nc = _build_module()
    trace = _os.environ.get("KERNEL_TRACE", "0") == "1"
    if trace:
        _install_ntff_hook()
    res = bass_utils.run_bass_kernel_spmd(
        nc, in_maps, core_ids=list(range(N_CORES)), trace=trace,
    )
    LAST_EXEC_NS = res.exec_time_ns
    parts = [res.results[n]["part"] for n in range(N_CORES)]
    return combine_partials(parts)


# revision 17
# speedup vs baseline: 1.2924x; 1.1268x over previous
"""MultiBox loss kernel for Trainium2 (Bass/Tile).

Layout: per core, one sample n. Priors padded 8732 -> 8832 = 128*69.
Prior p lives at (partition q = p // 69, column i = p % 69).
Dense tiles are (128, 1380) "i-major": free index i*20 + c.
Box-broadcast tiles are (128, 320) c-major: free index c*16 + m.

Match score: d = ln(inter) - ln(S') with S' = (areaA+areaB)*e^-SIG, so
d = ln(inter/S) + SIG.  iou >= 0.5  <=>  d >= SIG + ln(1/3).
Packing: qm = (d & ~0x7FF) | (15-m); QPA accumulates (qm | 16*(68-i))
max over i.  DVE STT ops (2x_2p mode, 0.5 cyc/el fp32) carry most of
the elementwise work; bitwise ORs of raw-bit codes ride STT scalars
(the STT scalar path preserves arbitrary bit patterns).
"""
import numpy as np

import concourse.bass as bass
import concourse.mybir as mybir
from concourse import tile
from concourse.alu_op_type import AluOpType
from concourse.bass import IndirectOffsetOnAxis

# ---------------- constants ----------------
C, P, M = 20, 8732, 16
QP, I = 128, 69           # partitions x columns
PP = QP * I               # 8832
CM = C * M                # 320
IC = I * C                # 1380
NEG_POS_RATIO = 3.0
SIG = 4.6                 # score shift
ESIG = float(np.exp(-SIG))
_thr = np.float32(np.float32(np.log(np.float32(1.0 / 3.0))) + np.float32(SIG))
THRP = float(np.int32(int(_thr.view(np.int32)) & ~0x7FF).view(np.float32))
SEL_ROWS, SEL_F = 80, 2208   # selection layout: 4 partitions x (69*32) per class
BISECT_ITERS = 10
DUMP_OFF = 10_000_000     # out-of-bounds scatter offset (dropped)
LN_MIN, LN_RANGE = -15.2, 9.3   # range of 5*ln(w) for box sizes

F32 = mybir.dt.float32
I32 = mybir.dt.int32
AF = mybir.ActivationFunctionType
AX = mybir.AxisListType

# ---------------- custom DVE ops ----------------
_REGISTERED = {}


def _register_op(name, spec, subdim=False):
    if name in _REGISTERED:
        return _REGISTERED[name]
    from concourse.dve_ops import DveOp, OPS, CUSTOM_DVE_SPECS, _SUB_OPCODE_FOR_NAME, _CUSTOM_DVE_ROW_BASE
    from concourse.dve_spec import lower, _has_src1
    from concourse.dve_uop import DveOpSpec
    row = _CUSTOM_DVE_ROW_BASE + len(OPS)
    assert row < 0x20
    _SUB_OPCODE_FOR_NAME[name] = row
    shas = {}
    for ver in ("v3", "v4"):
        s = DveOpSpec(name=name, opcode=row, uops=lower(spec, ver=ver), rd1_en=_has_src1(spec))
        shas[ver] = s.sha(ver)
    op = DveOp(name, spec, subdim=subdim, uops_sha=shas)
    OPS.append(op)
    CUSTOM_DVE_SPECS[name] = spec
    _REGISTERED[name] = op
    return op


def get_ops():
    from concourse.dve_spec import (Spec, Src0, Src1, C0, C1, C2, Zero,
                                    maxx, minn, select, AluOp, Idx, Bin)

    ovl = _register_op("ANT_OVL", Spec(
        body=maxx(minn(Src0, C0) - maxx(Src1, C1), C2),
        reference=lambda in0, in1, s0, s1, imm2: np.maximum(
            np.minimum(in0, s0) - np.maximum(in1, s1), imm2).astype(np.float32),
    ))

    def _idxmax_ref(in0, in1, s0, s1, imm2):
        n = in0.shape[1]
        out = np.where(in0 >= s0, s1 - np.arange(n)[None, :], 0.0).astype(np.float32)
        return out, out.max(axis=1, keepdims=True)

    idxmax = _register_op("ANT_IDXMAX", Spec(
        body=select(Src0 >= C0, C1 - Idx, Zero),
        accum=AluOp.MAX,
        reference=_idxmax_ref,
    ))

    def _selmax_ref(in0, in1, s0, s1, imm2):
        out = np.where(in0 >= s0, in1, 0.0).astype(np.float32)
        return out, out.max(axis=1, keepdims=True)

    selmax = _register_op("ANT_SELMAX", Spec(
        body=select(Src0 >= C0, Src1, Zero),
        accum=AluOp.MAX,
        reference=_selmax_ref,
    ))

    absd = _register_op("ANT_ABSD", Spec(
        body=Bin(AluOp.ABSOLUTE_DIFF, Src0, Src1),
        reference=lambda in0, in1, s0, s1, imm2: np.abs(in0 - in1).astype(np.float32),
    ))

    absds = _register_op("ANT_ABSDS", Spec(
        body=Bin(AluOp.ABSOLUTE_DIFF, Src0, Src1 * C0),
        reference=lambda in0, in1, s0, s1, imm2: np.abs(in0 - in1 * s0).astype(np.float32),
    ))

    def _sumgt_ref(in0, in1, s0, s1, imm2):
        out = np.where(in0 > s0, in0, 0.0).astype(np.float32)
        return out, out.sum(axis=1, keepdims=True, dtype=np.float32)

    sumgt = _register_op("ANT_SUMGT", Spec(
        body=select(Src0 > C0, Src0, Zero),
        accum=AluOp.ADD,
        reference=_sumgt_ref,
    ))
    return ovl, idxmax, selmax, sumgt, absd, absds


# ---------------- host-side input prep ----------------
def prep_core_inputs(scores_nc, locs_nc, boxes_nc):
    sc = np.zeros((C, QP * 138), np.float32)
    sc[:, : P * 2] = scores_nc.reshape(C, P * 2)
    lc = np.zeros((C, QP * 276), np.float32)
    lc[:, : P * 4] = locs_nc.reshape(C, P * 4)
    return {
        "scores_pad": sc,
        "locs_pad": lc,
        "boxes_t": boxes_nc.reshape(1, CM * 4).astype(np.float32),
    }


def prep_shared_inputs(priors):
    pr = np.zeros((PP, 4), np.float32)
    pr[:P] = priors
    pr[P:, 0] = 50.0 + np.arange(PP - P)
    pr[P:, 1] = 50.0
    pr[P:, 2] = 0.01
    pr[P:, 3] = 0.01

    ident = np.eye(QP, dtype=np.float32)
    ind120 = np.zeros((SEL_ROWS, C), np.float32)
    for k in range(SEL_ROWS):
        ind120[k, k // 4] = 1.0
    indT = np.ascontiguousarray(ind120.T)
    later = np.zeros((QP, QP), np.float32)
    for a in range(QP):
        for b in range(QP):
            if b > a and b // M == a // M:
                later[a, b] = 1.0
    pidx = np.arange(QP)[:, None] * I + np.arange(I)[None, :]   # (128, 69)
    padmask = (pidx < P).astype(np.float32)[:, :, None].repeat(C, 2).reshape(QP, IC)
    part = np.arange(QP)
    coffs = np.stack([((b * QP + part) // M).astype(np.float32) for b in range(3)], 1)
    mvals = np.stack([(15.0 - (b * QP + part) % M).astype(np.float32) for b in range(3)], 1)
    # per-column packed codes: (15-m) | 16*(68-i), partition-invariant
    iidx = np.arange(I)
    cm_m = np.arange(CM) % M
    codes = ((16 * (68 - iidx))[:, None] | (15 - cm_m)[None, :]).astype(np.int32)  # (I, CM)
    mdcol = np.broadcast_to(codes.reshape(1, I * CM), (QP, I * CM))
    mdcol = np.ascontiguousarray(mdcol).view(np.float32)
    return {
        "mdcol": mdcol,
        "priors_t": pr,
        "ident": ident,
        "ind120": ind120,
        "indT": indT,
        "later": later,
        "coffs": coffs,
        "mvals": mvals,
        "padmask": padmask,
    }


# ---------------- the kernel ----------------
def build_kernel(tc, outs, ins):
    nc = tc.nc
    OVL, IDXMAX, SELMAX, SUMGT, ABSD, ABSDS = get_ops()

    out_part = outs["part"]      # (8, 20) f32

    from contextlib import ExitStack
    with ExitStack() as ctx:
        cpool = ctx.enter_context(tc.tile_pool(name="const", bufs=1))
        lpool = ctx.enter_context(tc.tile_pool(name="loop", bufs=2))
        ppool = ctx.enter_context(tc.tile_pool(name="psum", bufs=2, space="PSUM"))
        dpool = ctx.enter_context(tc.tile_pool(name="dram", bufs=1, space="DRAM"))
        _build(nc, tc, cpool, lpool, ppool, dpool, ins, out_part,
               OVL, IDXMAX, SELMAX, SUMGT, ABSD, ABSDS)


def _build(nc, tc, cpool, lpool, ppool, dpool, ins, out_part, OVL, IDXMAX, SELMAX, SUMGT, ABSD, ABSDS):
    scores = ins["scores_pad"]
    locs = ins["locs_pad"]
    boxes_t = ins["boxes_t"]
    priors_t = ins["priors_t"]
    stt = nc.vector.scalar_tensor_tensor

    # ---- load constants / inputs ----
    PR = cpool.tile([QP, I, 4], F32)
    nc.sync.dma_start(out=PR[:], in_=priors_t.rearrange("(q i) k -> q i k", q=QP))
    IDENT = cpool.tile([QP, QP], F32)
    nc.sync.dma_start(out=IDENT[:], in_=ins["ident"])
    IND120 = cpool.tile([SEL_ROWS, C], F32)
    nc.sync.dma_start(out=IND120[:], in_=ins["ind120"])
    INDT = cpool.tile([C, SEL_ROWS], F32)
    nc.sync.dma_start(out=INDT[:], in_=ins["indT"])
    LATER = cpool.tile([QP, QP], F32)
    nc.sync.dma_start(out=LATER[:], in_=ins["later"])

    SC = cpool.tile([QP, C, 138], F32)
    nc.sync.dma_start(out=SC[:], in_=scores.rearrange("c (q e) -> q c e", q=QP))
    PL = cpool.tile([QP, C, 276], F32)
    nc.sync.dma_start(out=PL[:], in_=locs.rearrange("c (q e) -> q c e", q=QP))
    BT = cpool.tile([1, CM * 4], F32)
    nc.sync.dma_start(out=BT[:], in_=boxes_t)

    PADM = cpool.tile([QP, IC], F32, tag="padm")
    nc.sync.dma_start(out=PADM[:], in_=ins["padmask"])
    CONSTI = cpool.tile([QP, 9], I32)
    # 0: pack mask ~0x7FF, 1: col extract 0x7F0, 2: m extract 0xF,
    # 3: 63, 4: 0xFC0, 5: 0x3F000, 6: 0xFC0000, 7: unused, 8: 0
    for _k, _v in enumerate([~0x7FF, 0x7F0, 0xF, 63, 0xFC0, 0x3F000, 0xFC0000, 0, 0]):
        nc.vector.memset(CONSTI[:, _k:_k + 1], _v)
    COFF = cpool.tile([QP, 3], F32)
    nc.sync.dma_start(out=COFF[:], in_=ins["coffs"])
    VALS = cpool.tile([QP, 3], F32)
    nc.sync.dma_start(out=VALS[:], in_=ins["mvals"])

    # ---- prior-derived tiles (128, 69) ----
    pcx = PR[:, :, 0]
    pcy = PR[:, :, 1]
    pw = PR[:, :, 2]
    ph = PR[:, :, 3]
    PX1 = cpool.tile([QP, I], F32)
    PX2 = cpool.tile([QP, I], F32)
    PY1 = cpool.tile([QP, I], F32)
    PY2 = cpool.tile([QP, I], F32)
    PAREA = cpool.tile([QP, I], F32)   # prior area pre-scaled by exp(-SIG)
    stt(out=PX1[:], in0=pw, scalar=-0.5, in1=pcx, op0=AluOpType.mult, op1=AluOpType.add)
    stt(out=PX2[:], in0=pw, scalar=0.5, in1=pcx, op0=AluOpType.mult, op1=AluOpType.add)
    stt(out=PY1[:], in0=ph, scalar=-0.5, in1=pcy, op0=AluOpType.mult, op1=AluOpType.add)
    stt(out=PY2[:], in0=ph, scalar=0.5, in1=pcy, op0=AluOpType.mult, op1=AluOpType.add)
    stt(out=PAREA[:], in0=pw, scalar=ESIG, in1=ph, op0=AluOpType.mult, op1=AluOpType.mult)
    IPW = cpool.tile([QP, I], F32)   # 10 / pw
    IPH = cpool.tile([QP, I], F32)
    scr69 = cpool.tile([QP, I], F32)
    nc.vector.reciprocal_approx_accurate(out=IPW[:], in_=pw, scratch=scr69[:])
    nc.vector.reciprocal_approx_accurate(out=IPH[:], in_=ph, scratch=scr69[:])
    nc.vector.tensor_scalar_mul(IPW[:], IPW[:], 10.0)
    nc.vector.tensor_scalar_mul(IPH[:], IPH[:], 10.0)
    LPW5 = cpool.tile([QP, I], F32)  # 5*ln(pw)
    LPH5 = cpool.tile([QP, I], F32)
    nc.scalar.activation(out=LPW5[:], in_=pw, func=AF.Ln)
    nc.scalar.activation(out=LPH5[:], in_=ph, func=AF.Ln)
    nc.vector.tensor_scalar_mul(LPW5[:], LPW5[:], 5.0)
    nc.vector.tensor_scalar_mul(LPH5[:], LPH5[:], 5.0)

    # per-prior l1 helper tiles (no loop deps; emitted early for overlap)
    IPW63 = cpool.tile([QP, I], F32)
    IPH63 = cpool.tile([QP, I], F32)
    nc.vector.tensor_scalar_mul(IPW63[:], IPW[:], 1.0 / 63.0)
    nc.vector.tensor_scalar_mul(IPH63[:], IPH[:], 1.0 / (63.0 * 64.0))
    PCXI = cpool.tile([QP, I], F32)
    PCYI = cpool.tile([QP, I], F32)
    nc.vector.tensor_tensor(out=PCXI[:], in0=pcx, in1=IPW[:], op=AluOpType.mult)
    nc.vector.tensor_tensor(out=PCYI[:], in0=pcy, in1=IPH[:], op=AluOpType.mult)

    # ---- box-derived broadcast tiles (128, 320) c-major ----
    bx1v = BT[:, 0::4]
    by1v = BT[:, 1::4]
    bx2v = BT[:, 2::4]
    by2v = BT[:, 3::4]
    BD = cpool.tile([1, CM * 10], F32, tag="cbslot")
    s = [BD[:, k * CM:(k + 1) * CM] for k in range(10)]
    # 0 bx1, 1 bx2, 2 by1, 3 by2, 4 bcx, 5 bcy, 6 areab*e^-SIG, 7 lnbw5, 8 lnbh5, 9 quad
    nc.vector.tensor_copy(out=s[0], in_=bx1v)
    nc.vector.tensor_copy(out=s[1], in_=bx2v)
    nc.vector.tensor_copy(out=s[2], in_=by1v)
    nc.vector.tensor_copy(out=s[3], in_=by2v)
    t1 = cpool.tile([1, CM], F32)
    nc.vector.tensor_tensor(out=t1[:], in0=bx1v, in1=bx2v, op=AluOpType.add)
    nc.vector.tensor_scalar_mul(s[4], t1[:], 0.5)
    nc.vector.tensor_tensor(out=t1[:], in0=by1v, in1=by2v, op=AluOpType.add)
    nc.vector.tensor_scalar_mul(s[5], t1[:], 0.5)
    tbw = cpool.tile([1, CM], F32)
    tbh = cpool.tile([1, CM], F32)
    nc.vector.tensor_tensor(out=tbw[:], in0=bx2v, in1=bx1v, op=AluOpType.subtract)
    nc.vector.tensor_tensor(out=tbh[:], in0=by2v, in1=by1v, op=AluOpType.subtract)
    stt(out=s[6], in0=tbw[:], scalar=ESIG, in1=tbh[:], op0=AluOpType.mult, op1=AluOpType.mult)
    nc.scalar.activation(out=s[7], in_=tbw[:], func=AF.Ln)
    nc.scalar.activation(out=s[8], in_=tbh[:], func=AF.Ln)
    nc.vector.tensor_scalar_mul(s[7], s[7], 5.0)
    nc.vector.tensor_scalar_mul(s[8], s[8], 5.0)
    # quad code: round(bcx*63) + 64*round(bcy*63) + 64^2*round((lnw5-LN_MIN)*63/LN_RANGE) + 64^3*(...)
    enc = cpool.tile([1, CM], I32)
    encf = cpool.tile([1, CM], F32)
    quad = s[9]
    nc.vector.tensor_scalar(out=enc[:], in0=s[4], scalar1=63.0, scalar2=0.5,
                            op0=AluOpType.mult, op1=AluOpType.add)
    nc.vector.tensor_copy(out=quad, in_=enc[:])
    nc.vector.tensor_scalar(out=enc[:], in0=s[5], scalar1=63.0, scalar2=0.5,
                            op0=AluOpType.mult, op1=AluOpType.add)
    nc.vector.tensor_copy(out=encf[:], in_=enc[:])
    stt(out=quad, in0=encf[:], scalar=64.0, in1=quad, op0=AluOpType.mult, op1=AluOpType.add)
    nc.vector.tensor_scalar(out=enc[:], in0=s[7], scalar1=63.0 / LN_RANGE,
                            scalar2=-LN_MIN * 63.0 / LN_RANGE + 0.5,
                            op0=AluOpType.mult, op1=AluOpType.add)
    nc.vector.tensor_copy(out=encf[:], in_=enc[:])
    stt(out=quad, in0=encf[:], scalar=4096.0, in1=quad, op0=AluOpType.mult, op1=AluOpType.add)
    nc.vector.tensor_scalar(out=enc[:], in0=s[8], scalar1=63.0 / LN_RANGE,
                            scalar2=-LN_MIN * 63.0 / LN_RANGE + 0.5,
                            op0=AluOpType.mult, op1=AluOpType.add)
    nc.vector.tensor_copy(out=encf[:], in_=enc[:])
    stt(out=quad, in0=encf[:], scalar=262144.0, in1=quad, op0=AluOpType.mult, op1=AluOpType.add)

    ONES1 = cpool.tile([1, QP], F32)
    nc.vector.memset(ONES1[:], 1.0)
    BB = cpool.tile([QP, CM * 10], F32)
    tot = CM * 10
    off = 0
    while off < tot:
        w = min(512, tot - off)
        pt = ppool.tile([QP, w], F32, tag="bcast")
        nc.tensor.matmul(out=pt[:], lhsT=ONES1[:], rhs=BD[:, off:off + w], start=True, stop=True)
        nc.scalar.copy(out=BB[:, off:off + w], in_=pt[:])
        off += w
    BX1 = BB[:, 0 * CM:1 * CM]
    BX2 = BB[:, 1 * CM:2 * CM]
    BY1 = BB[:, 2 * CM:3 * CM]
    BY2 = BB[:, 3 * CM:4 * CM]
    BAR = BB[:, 6 * CM:7 * CM]
    QUADB = BB[:, 9 * CM:10 * CM]

    # ---- CE (no dependency on matching; emitted early for engine overlap) ----
    DM = cpool.tile([QP, IC], F32, tag="dm")
    sc4 = SC[:].rearrange("p c (i two) -> p c i two", two=2)
    dm3 = DM[:].rearrange("p (i c) -> p i c", c=C)
    stt(out=dm3.rearrange("p i c -> p c i"), in0=sc4[:, :, :, 1], scalar=1.0,
        in1=sc4[:, :, :, 0], op0=AluOpType.mult, op1=AluOpType.subtract)
    CE = cpool.tile([QP, IC], F32)
    nc.scalar.activation(out=CE[:], in_=DM[:], func=AF.Exp)
    nc.scalar.activation(out=CE[:], in_=CE[:], func=AF.Ln, bias=1.0)

    # init FMD scratch early (no deps)
    FMD = dpool.tile([PP * C, 1], F32)
    NEG1 = cpool.tile([QP, IC], F32, tag="l1a")
    nc.vector.memset(NEG1[:], -1.0)
    nc.sync.dma_start(out=FMD[:].rearrange("(q f) one -> q (f one)", q=QP), in_=NEG1[:])

    # ---- accumulators ----
    QMM = cpool.tile([QP, I, C], F32)
    QPA = cpool.tile([QP, CM], F32)
    nc.vector.memset(QPA[:], 0.0)
    mdcol_d = ins["mdcol"]

    # ================= main loop over columns i =================
    MCH = 4   # mdcol prefetch chunk
    for i in range(I):
        if i % MCH == 0:
            nch = min(MCH, I - i)
            MDCOL = lpool.tile([QP, MCH * CM], F32, tag="mdcol")
            nc.sync.dma_start(out=MDCOL[:, :nch * CM],
                              in_=mdcol_d[:, i * CM:(i + nch) * CM])
        xov = lpool.tile([QP, CM], F32, tag="xov")
        nc.vector._custom_dve(OVL, out=xov[:], in0=BX2, in1=BX1,
                              s0=PX2[:, i:i + 1], s1=PX1[:, i:i + 1], imm2=1e-18)
        yov = lpool.tile([QP, CM], F32, tag="yov")
        nc.vector._custom_dve(OVL, out=yov[:], in0=BY2, in1=BY1,
                              s0=PY2[:, i:i + 1], s1=PY1[:, i:i + 1], imm2=1e-18)
        inter = lpool.tile([QP, CM], F32, tag="inter")
        stt(out=inter[:], in0=xov[:], scalar=1.0, in1=yov[:],
            op0=AluOpType.mult, op1=AluOpType.mult)
        S = lpool.tile([QP, CM], F32, tag="S")
        nc.scalar.activation(out=S[:], in_=BAR, func=AF.Identity,
                             bias=PAREA[:, i:i + 1], scale=1.0)
        lnI = lpool.tile([QP, CM], F32, tag="lnI")
        nc.scalar.activation(out=lnI[:], in_=inter[:], func=AF.Ln)
        lnS = lpool.tile([QP, CM], F32, tag="lnS")
        nc.scalar.activation(out=lnS[:], in_=S[:], func=AF.Ln)
        d = lpool.tile([QP, CM], F32, tag="d")
        stt(out=d[:], in0=lnI[:], scalar=1.0, in1=lnS[:],
            op0=AluOpType.mult, op1=AluOpType.subtract)
        if i % 2 == 0:
            QM2 = lpool.tile([QP, 2 * CM], F32, tag="qm2")
        qmv = QM2[:, (i % 2) * CM:(i % 2 + 1) * CM]
        stt(out=qmv.bitcast(I32), in0=d[:].bitcast(I32), scalar=CONSTI[:, 0:1],
            in1=MDCOL[:, (i % MCH) * CM:(i % MCH + 1) * CM].bitcast(I32),
            op0=AluOpType.bitwise_and, op1=AluOpType.bitwise_or)
        stt(out=QPA[:], in0=qmv, scalar=1.0, in1=QPA[:],
            op0=AluOpType.mult, op1=AluOpType.max)
        if i % 2 == 1:
            nc.vector.tensor_reduce(
                out=QMM[:, i - 1:i + 1, :],
                in_=QM2[:].rearrange("p (x m) -> p x m", m=M),
                axis=AX.X, op=AluOpType.max)
        elif i == I - 1:
            nc.vector.tensor_reduce(
                out=QMM[:, i, :],
                in_=qmv.rearrange("p (c m) -> p c m", m=M),
                axis=AX.X, op=AluOpType.max)

    QMMf = QMM[:].rearrange("p i c -> p (i c)")
    QMMi = QMMf.bitcast(I32)

    # ================= pos mask, m* =================
    POSB = cpool.tile([QP, IC], F32, tag="posb")
    nc.vector.tensor_scalar(out=POSB[:], in0=QMMf, scalar1=THRP, scalar2=0.0,
                            op0=AluOpType.is_ge, op1=AluOpType.max)
    # m-code (15-m) in low 4 bits
    MSI = cpool.tile([QP, IC], I32, tag="ic_int")
    stt(out=MSI[:], in0=QMMi, scalar=CONSTI[:, 2:3],
        in1=CONSTI[:, 8:9].to_broadcast([QP, IC]),
        op0=AluOpType.bitwise_and, op1=AluOpType.bitwise_or)
    MS = cpool.tile([QP, IC], F32)
    nc.vector.tensor_copy(out=MS[:], in_=MSI[:])

    # ================= prior_for_obj (forced positives) =================
    QPAf = QPA[:]
    PSTARI = cpool.tile([QP, 3], I32)
    for b in range(3):
        w = 128 if b < 2 else 64
        tp = ppool.tile([QP, QP], F32, tag="ptr")
        nc.tensor.transpose(out=tp[:w, :], in_=QPAf[:, b * QP:b * QP + w], identity=IDENT[:])
        TQ = lpool.tile([QP, QP], F32, tag="TQ")
        nc.scalar.copy(out=TQ[:w, :], in_=tp[:w, :])
        vmax = lpool.tile([QP, 1], F32, tag="vmax")
        nc.vector.tensor_reduce(out=vmax[:w], in_=TQ[:w, :], axis=AX.X, op=AluOpType.max)
        qd = lpool.tile([QP, 1], F32, tag="qd")
        sc1 = lpool.tile([QP, QP], F32, tag="sc1")
        nc.vector._custom_dve(IDXMAX, out=sc1[:w, :], accum_out=qd[:w], in0=TQ[:w, :],
                              s0=vmax[:w], s1=127.0)
        TLI = lpool.tile([QP, QP], I32, tag="TLI")
        stt(out=TLI[:w, :], in0=TQ[:w, :].bitcast(I32), scalar=CONSTI[:w, 1:2],
            in1=CONSTI[:w, 8:9].to_broadcast([w, QP]),
            op0=AluOpType.bitwise_and, op1=AluOpType.bitwise_or)
        TLF = lpool.tile([QP, QP], F32, tag="TLF")
        nc.vector.tensor_copy(out=TLF[:w, :], in_=TLI[:w, :])
        colv = lpool.tile([QP, 1], F32, tag="ilow")
        sc2 = lpool.tile([QP, QP], F32, tag="sc2")
        nc.vector._custom_dve(SELMAX, out=sc2[:w, :], accum_out=colv[:w], in0=TQ[:w, :],
                              in1=TLF[:w, :], s0=vmax[:w])
        # p* = (127 - qd)*69 + (68 - colv/16)
        pst = lpool.tile([QP, 1], F32, tag="pst")
        nc.vector.tensor_scalar(out=pst[:w], in0=qd[:w], scalar1=-69.0,
                                scalar2=float(127 * 69 + 68),
                                op0=AluOpType.mult, op1=AluOpType.add)
        stt(out=pst[:w], in0=colv[:w], scalar=-1.0 / 16.0, in1=pst[:w],
            op0=AluOpType.mult, op1=AluOpType.add)
        # dedup: later m with same p* in same class wins
        tpp = ppool.tile([QP, QP], F32, tag="ptr")
        nc.tensor.transpose(out=tpp[:, :w], in_=pst[:w, :1].to_broadcast([w, QP]),
                            identity=IDENT[:w, :w])
        PTT = lpool.tile([QP, QP], F32, tag="PTT")
        nc.scalar.copy(out=PTT[:, :w], in_=tpp[:, :w])
        EQM = lpool.tile([QP, QP], F32, tag="EQM")
        nc.vector.tensor_tensor(out=EQM[:w, :w], in0=pst[:w, :1].to_broadcast([w, w]),
                                in1=PTT[:w, :w], op=AluOpType.is_equal)
        nc.vector.tensor_tensor(out=EQM[:w, :w], in0=EQM[:w, :w], in1=LATER[:w, :w],
                                op=AluOpType.mult)
        dom = lpool.tile([QP, 1], F32, tag="dom")
        nc.vector.tensor_reduce(out=dom[:w], in_=EQM[:w, :w], axis=AX.X, op=AluOpType.max)
        # offset = p* * 20 + c; dominated -> +DUMP_OFF (dropped by bounds check)
        offf = lpool.tile([QP, 1], F32, tag="offf")
        stt(out=offf[:w], in0=pst[:w], scalar=20.0, in1=COFF[:w, b:b + 1],
            op0=AluOpType.mult, op1=AluOpType.add)
        stt(out=offf[:w], in0=dom[:w], scalar=float(DUMP_OFF), in1=offf[:w],
            op0=AluOpType.mult, op1=AluOpType.add)
        nc.vector.tensor_copy(out=PSTARI[:w, b:b + 1], in_=offf[:w])

    for b in range(3):
        w = 128 if b < 2 else 64
        nc.gpsimd.indirect_dma_start(
            out=FMD[:],
            out_offset=IndirectOffsetOnAxis(ap=PSTARI[:w, b:b + 1], axis=0),
            in_=VALS[:w, b:b + 1],
            in_offset=None,
            bounds_check=PP * C - 1,
            oob_is_err=False,
        )
    FM = cpool.tile([QP, IC], F32, tag="fm")
    nc.sync.dma_start(out=FM[:], in_=FMD[:].rearrange("(q f) one -> q (f one)", q=QP))

    FGE = cpool.tile([QP, IC], F32)
    nc.vector.tensor_scalar(out=FGE[:], in0=FM[:], scalar1=0.0, scalar2=0.0,
                            op0=AluOpType.is_ge, op1=AluOpType.max)
    POSB2 = POSB
    nc.vector.tensor_tensor(out=POSB2[:], in0=POSB[:], in1=FGE[:], op=AluOpType.max)
    FGEI = cpool.tile([QP, IC], I32, tag="ic_int")
    nc.vector.tensor_copy(out=FGEI[:], in_=FGE[:])
    MS2 = MS
    nc.vector.copy_predicated(out=MS2[:], mask=FGEI[:], data=FM[:])

    # ================= CE pos/neg splits =================
    CEP = cpool.tile([QP, IC], F32, tag="cep")
    stt(out=CEP[:], in0=PADM[:], scalar=1.0, in1=POSB2[:],
        op0=AluOpType.mult, op1=AluOpType.subtract)
    CEN = cpool.tile([QP, C, I], F32, tag="scbslot")
    cen_im = CEN[:].rearrange("p c i -> p i c")
    stt(out=cen_im, in0=CE[:].rearrange("p (i c) -> p i c", c=C), scalar=1.0,
        in1=CEP[:].rearrange("p (i c) -> p i c", c=C),
        op0=AluOpType.mult, op1=AluOpType.mult)
    CPT = cpool.tile([QP, IC], F32, tag="gt")
    stt(out=CPT[:], in0=CE[:], scalar=1.0, in1=DM[:],
        op0=AluOpType.mult, op1=AluOpType.subtract)
    stt(out=CPT[:], in0=CPT[:], scalar=1.0, in1=POSB2[:],
        op0=AluOpType.mult, op1=AluOpType.mult)

    # ================= counts / class sums =================
    NPQ = cpool.tile([QP, C], F32)
    nc.vector.tensor_reduce(out=NPQ[:], in_=POSB2[:].rearrange("p (i c) -> p c i", c=C),
                            axis=AX.X, op=AluOpType.add)
    CPQ = cpool.tile([QP, C], F32)
    nc.vector.tensor_reduce(out=CPQ[:], in_=CPT[:].rearrange("p (i c) -> p c i", c=C),
                            axis=AX.X, op=AluOpType.add)
    ONESC = cpool.tile([QP, 1], F32)
    nc.vector.memset(ONESC[:], 1.0)
    NPC_p = ppool.tile([1, C], F32, tag="pmm")
    nc.tensor.matmul(out=NPC_p[:], lhsT=ONESC[:], rhs=NPQ[:], start=True, stop=True)
    CPC_p = ppool.tile([1, C], F32, tag="pmm")
    nc.tensor.matmul(out=CPC_p[:], lhsT=ONESC[:], rhs=CPQ[:], start=True, stop=True)
    NPC = cpool.tile([1, C], F32)
    nc.scalar.copy(out=NPC[:], in_=NPC_p[:])
    CPC = cpool.tile([1, C], F32)
    nc.scalar.copy(out=CPC[:], in_=CPC_p[:])

    kp = ppool.tile([C, 1], F32, tag="pmm")
    nc.tensor.transpose(out=kp[:], in_=NPC[:], identity=IDENT[:1, :1])
    KC = cpool.tile([C, 1], F32)
    nc.scalar.copy(out=KC[:], in_=kp[:])
    nc.vector.tensor_scalar_mul(KC[:], KC[:], NEG_POS_RATIO)

    # ================= hard-negative selection =================
    CB = cpool.tile([SEL_ROWS, SEL_F], F32, tag="cbslot")
    for c in range(C):
        nc.sync.dma_start(out=CB[c * 4:(c + 1) * 4, :], in_=CEN[:, c, :])

    LO = cpool.tile([C, 1], F32)
    HI = cpool.tile([C, 1], F32)
    TC_ = cpool.tile([C, 1], F32)
    nc.vector.memset(LO[:], 0.8)
    nc.vector.memset(HI[:], 4.0)
    T120 = cpool.tile([SEL_ROWS, 1], F32)
    CNT6 = cpool.tile([SEL_ROWS, 1], F32)
    CNTC = cpool.tile([C, 1], F32)
    scb = cpool.tile([SEL_ROWS, SEL_F], F32, tag="scbslot")
    for it in range(BISECT_ITERS):
        nc.vector.tensor_tensor(out=TC_[:], in0=LO[:], in1=HI[:], op=AluOpType.add)
        nc.vector.tensor_scalar_mul(TC_[:], TC_[:], 0.5)
        tp120 = ppool.tile([SEL_ROWS, 1], F32, tag="pmm")
        nc.tensor.matmul(out=tp120[:], lhsT=INDT[:], rhs=TC_[:], start=True, stop=True)
        nc.scalar.copy(out=T120[:], in_=tp120[:])
        nc.vector.tensor_scalar(out=scb[:], in0=CB[:], scalar1=T120[:, :1], scalar2=0.0,
                                op0=AluOpType.is_gt, op1=AluOpType.add, accum_out=CNT6[:])
        tpc = ppool.tile([C, 1], F32, tag="pmm")
        nc.tensor.matmul(out=tpc[:], lhsT=IND120[:], rhs=CNT6[:], start=True, stop=True)
        nc.scalar.copy(out=CNTC[:], in_=tpc[:])
        gm = lpool.tile([C, 1], I32, tag="gm")
        nc.vector.tensor_tensor(out=gm[:], in0=CNTC[:], in1=KC[:], op=AluOpType.is_ge)
        nc.vector.copy_predicated(out=LO[:], mask=gm[:], data=TC_[:])
        lm = lpool.tile([C, 1], I32, tag="lm")
        nc.vector.tensor_tensor(out=lm[:], in0=CNTC[:], in1=KC[:], op=AluOpType.is_lt)
        nc.vector.copy_predicated(out=HI[:], mask=lm[:], data=TC_[:])
    tp120 = ppool.tile([SEL_ROWS, 1], F32, tag="pmm")
    nc.tensor.matmul(out=tp120[:], lhsT=INDT[:], rhs=LO[:], start=True, stop=True)
    nc.scalar.copy(out=T120[:], in_=tp120[:])
    SUM6 = cpool.tile([SEL_ROWS, 1], F32)
    nc.vector._custom_dve(SUMGT, out=scb[:], accum_out=SUM6[:], in0=CB[:], s0=T120[:, :1])
    nc.vector.tensor_scalar(out=scb[:], in0=CB[:], scalar1=T120[:, :1], scalar2=0.0,
                            op0=AluOpType.is_gt, op1=AluOpType.add, accum_out=CNT6[:])
    SUMC_p = ppool.tile([C, 1], F32, tag="pmm")
    nc.tensor.matmul(out=SUMC_p[:], lhsT=IND120[:], rhs=SUM6[:], start=True, stop=True)
    CNTC_p = ppool.tile([C, 1], F32, tag="pmm")
    nc.tensor.matmul(out=CNTC_p[:], lhsT=IND120[:], rhs=CNT6[:], start=True, stop=True)
    CH = cpool.tile([C, 1], F32)
    nc.scalar.copy(out=CNTC[:], in_=CNTC_p[:])
    nc.vector.tensor_tensor(out=CH[:], in0=KC[:], in1=CNTC[:], op=AluOpType.subtract)
    nc.vector.tensor_tensor(out=CH[:], in0=CH[:], in1=LO[:], op=AluOpType.mult)
    SUMC = cpool.tile([C, 1], F32)
    nc.scalar.copy(out=SUMC[:], in_=SUMC_p[:])
    nc.vector.tensor_tensor(out=CH[:], in0=CH[:], in1=SUMC[:], op=AluOpType.add)

    # ================= localization loss =================
    # 16-way select of quad-encoded box quantities by m* code (15-m);
    # selects are disjoint so plain adds accumulate G.
    G = cpool.tile([QP, IC], F32, tag="gt")
    g3 = G[:].rearrange("p (i c) -> p i c", c=C)
    QUADC = cpool.tile([QP, CM], F32)   # m-major reorder for contiguous in1
    nc.vector.tensor_copy(out=QUADC[:].rearrange("p (m c) -> p m c", m=M),
                          in_=QUADB.rearrange("p (c m) -> p m c", m=M))

    def quadview(m):
        return QUADC[:, m * C:(m + 1) * C].unsqueeze(1).to_broadcast([QP, I, C])

    nc.vector.memset(G[:], 0.0)
    ms3 = MS2[:].rearrange("p (i c) -> p i c", c=C)
    TQM = cpool.tile([QP, I, C], F32, tag="tqm")
    tq3 = TQM[:]
    for m in range(M):
        stt(out=tq3, in0=ms3, scalar=float(15 - m), in1=quadview(m),
            op0=AluOpType.is_equal, op1=AluOpType.mult)
        stt(out=g3, in0=tq3, scalar=1.0, in1=g3, op0=AluOpType.mult, op1=AluOpType.add)
    GI = cpool.tile([QP, IC], I32, tag="ic_int")
    nc.vector.tensor_copy(out=GI[:], in_=G[:])

    L1A = cpool.tile([QP, IC], F32, tag="l1a")
    nc.vector.memset(L1A[:], 0.0)
    EC = cpool.tile([QP, IC], F32, tag="dm")
    ECI = cpool.tile([QP, IC], I32, tag="ec_int")
    TM2 = cpool.tile([QP, IC], F32, tag="cep")
    tm3 = TM2[:].rearrange("p (i c) -> p i c", c=C)
    ec3 = EC[:].rearrange("p (i c) -> p i c", c=C)
    pl5 = PL[:].rearrange("p c (i four) -> p c i four", four=4)

    def bc69(t):
        return t[:].unsqueeze(2).to_broadcast([QP, I, C])

    def l1_xy(mask_col, scale_t, pci_t, k_coord):
        stt(out=ECI[:], in0=GI[:], scalar=CONSTI[:, mask_col:mask_col + 1],
            in1=CONSTI[:, 8:9].to_broadcast([QP, IC]),
            op0=AluOpType.bitwise_and, op1=AluOpType.bitwise_or)
        nc.vector.tensor_copy(out=EC[:], in_=ECI[:])
        # A = pl + pcx*ipw ; t = e * (ipw/63/shift); diff = A - t
        plv = pl5[:, :, :, k_coord].rearrange("p c i -> p i c")
        stt(out=tm3, in0=plv, scalar=1.0, in1=bc69(pci_t),
            op0=AluOpType.mult, op1=AluOpType.add)
        stt(out=ec3, in0=ec3, scalar=1.0, in1=bc69(scale_t),
            op0=AluOpType.mult, op1=AluOpType.mult)
        nc.vector._custom_dve(ABSD, out=TM2[:], in0=TM2[:], in1=EC[:])
        stt(out=L1A[:], in0=TM2[:], scalar=1.0, in1=L1A[:],
            op0=AluOpType.mult, op1=AluOpType.add)

    l1_xy(3, IPW63, PCXI, 0)          # cx: e in [0,63], value e/63 * ipw
    l1_xy(4, IPH63, PCYI, 1)          # cy: e-bits at <<6; scale = iph/(63*64)

    # w/h coords: A = pl + lnpw5 - LN_MIN ; t = e * (LN_RANGE/63/shift)
    def l1_wh(mask_col, shift, lp5, k_coord):
        stt(out=ECI[:], in0=GI[:], scalar=CONSTI[:, mask_col:mask_col + 1],
            in1=CONSTI[:, 8:9].to_broadcast([QP, IC]),
            op0=AluOpType.bitwise_and, op1=AluOpType.bitwise_or)
        nc.vector.tensor_copy(out=EC[:], in_=ECI[:])
        plv = pl5[:, :, :, k_coord].rearrange("p c i -> p i c")
        stt(out=tm3, in0=plv, scalar=-LN_MIN, in1=bc69(lp5),
            op0=AluOpType.add, op1=AluOpType.add)
        nc.vector._custom_dve(ABSDS, out=TM2[:], in0=TM2[:], in1=EC[:],
                              s0=LN_RANGE / 63.0 / shift)
        stt(out=L1A[:], in0=TM2[:], scalar=1.0, in1=L1A[:],
            op0=AluOpType.mult, op1=AluOpType.add)

    l1_wh(5, 4096.0, LPW5, 2)
    l1_wh(6, 262144.0, LPH5, 3)

    stt(out=L1A[:], in0=L1A[:], scalar=1.0, in1=POSB2[:],
        op0=AluOpType.mult, op1=AluOpType.mult)
    L1Q = cpool.tile([QP, C], F32)
    nc.vector.tensor_reduce(out=L1Q[:], in_=L1A[:].rearrange("p (i c) -> p c i", c=C),
                            axis=AX.X, op=AluOpType.add)
    L1C_p = ppool.tile([1, C], F32, tag="pmm")
    nc.tensor.matmul(out=L1C_p[:], lhsT=ONESC[:], rhs=L1Q[:], start=True, stop=True)
    L1C = cpool.tile([1, C], F32)
    nc.scalar.copy(out=L1C[:], in_=L1C_p[:])

    # ================= outputs =================
    chp = ppool.tile([1, C], F32, tag="pmm")
    nc.tensor.transpose(out=chp[:], in_=CH[:, :1], identity=IDENT[:C, :C])
    CHR = cpool.tile([1, C], F32)
    nc.scalar.copy(out=CHR[:], in_=chp[:])
    nc.sync.dma_start(out=out_part[0:1, :], in_=NPC[:])
    nc.sync.dma_start(out=out_part[1:2, :], in_=CPC[:])
    nc.sync.dma_start(out=out_part[2:3, :], in_=CHR[:])
    nc.sync.dma_start(out=out_part[3:4, :], in_=L1C[:])


# ---------------- host reference partials (for validation) ----------------
def numpy_partials(scores_nc, locs_nc, boxes_nc, priors):
    def cxcy_to_xy(c):
        return np.concatenate([c[..., :2] - c[..., 2:] / 2, c[..., :2] + c[..., 2:] / 2], -1)

    priors_xy = cxcy_to_xy(priors)
    n_pos = np.zeros(C); conf_pos = np.zeros(C); conf_hard = np.zeros(C); l1s = np.zeros(C)
    for c in range(C):
        b = boxes_nc[c]
        lo = np.maximum(b[:, None, :2], priors_xy[None, :, :2])
        hi = np.minimum(b[:, None, 2:], priors_xy[None, :, 2:])
        inter = np.prod(np.clip(hi - lo, 0, None), -1)
        aa = np.prod(b[:, 2:] - b[:, :2], -1)
        ab = np.prod(priors_xy[:, 2:] - priors_xy[:, :2], -1)
        ov = (inter / (aa[:, None] + ab[None, :] - inter)).astype(np.float32)
        ofp = ov.argmax(0); vfp = ov.max(0)
        pfo = ov.argmax(1)
        ofp[pfo] = np.arange(M); vfp[pfo] = 1.0
        pos = vfp >= 0.5
        n_pos[c] = pos.sum()
        d = (scores_nc[c, :, 1] - scores_nc[c, :, 0]).astype(np.float32)
        ce = np.logaddexp(0, np.where(pos, -d, d)).astype(np.float32)
        conf_pos[c] = ce[pos].sum()
        ce_neg = np.where(pos, 0, ce)
        k = int(3 * n_pos[c])
        srt = np.sort(ce_neg)[::-1]
        conf_hard[c] = srt[:k].sum()
        bm = b[ofp]
        bcx = (bm[:, 0] + bm[:, 2]) / 2; bcy = (bm[:, 1] + bm[:, 3]) / 2
        bw = bm[:, 2] - bm[:, 0]; bh = bm[:, 3] - bm[:, 1]
        gcx = (bcx - priors[:, 0]) / (priors[:, 2] / 10)
        gcy = (bcy - priors[:, 1]) / (priors[:, 3] / 10)
        gw = np.log(bw / priors[:, 2]) * 5
        gh = np.log(bh / priors[:, 3]) * 5
        tl = np.stack([gcx, gcy, gw, gh], -1)
        l1 = np.abs(locs_nc[c] - tl).sum(-1) * pos
        l1s[c] = l1.sum()
    return np.stack([n_pos, conf_pos, conf_hard, l1s]).astype(np.float32)


def combine_partials(parts):
    tot = np.sum([p[:4] for p in parts], axis=0).astype(np.float64)
    n_pos_c, conf_pos_c, conf_hard_c, l1_c = tot
    loc_loss_c = l1_c / np.maximum(n_pos_c * 4.0, 1.0)
    safe = np.maximum(n_pos_c, 1.0)
    loss_c = np.where(n_pos_c > 0, (conf_pos_c + conf_hard_c + 1.0 * loc_loss_c) / safe, 0.0) / C
    return np.float32(loss_c.sum())


# ======================= entry point =======================
import os as _os

LAST_EXEC_NS = None
_COMPILED = None
N_CORES = 8


def _install_ntff_hook():
    """Provide antenv.axon_hooks if the image lacks it, so trace=True works."""
    import sys as _sys, types as _types
    try:
        from antenv.axon_hooks import get_axon_ntff_profile_hook  # noqa
        return
    except ImportError:
        pass
    mod = _types.ModuleType("antenv.axon_hooks")
    _h = {"hook": None}
    mod.set_axon_ntff_profile_hook = lambda h: _h.__setitem__("hook", h)
    mod.get_axon_ntff_profile_hook = lambda: _h["hook"]
    _sys.modules["antenv.axon_hooks"] = mod
    try:
        import antenv
        antenv.axon_hooks = mod
        from trn_agent_boot.trn_boot import _ntff_profile_via_ctypes
        mod.set_axon_ntff_profile_hook(_ntff_profile_via_ctypes("/opt/axon/libaxon_pjrt.so"))
    except Exception:
        pass


def _build_module():
    global _COMPILED
    if _COMPILED is not None:
        return _COMPILED
    import concourse.bacc as bacc
    from concourse.bass_interp import get_hw_module

    shapes = {
        "mdcol": (QP, I * CM),
        "scores_pad": (C, QP * 138),
        "locs_pad": (C, QP * 276),
        "boxes_t": (1, C * M * 4),
        "priors_t": (PP, 4),
        "ident": (QP, QP),
        "ind120": (SEL_ROWS, C),
        "indT": (C, SEL_ROWS),
        "later": (QP, QP),
        "coffs": (QP, 3),
        "mvals": (QP, 3),
        "padmask": (QP, IC),
    }
    nc = bacc.Bacc("TRN2", target_bir_lowering=False, debug=False, enable_asserts=False)
    in_aps = {}
    for name, shp in shapes.items():
        t = nc.dram_tensor(name, shp, mybir.dt.float32, kind="ExternalInput")
        in_aps[name] = t.ap()
    out_t = nc.dram_tensor("part", (8, C), mybir.dt.float32, kind="ExternalOutput")
    out_aps = {"part": out_t.ap()}
    with tile.TileContext(nc, trace_sim=False) as tc:
        build_kernel(tc, out_aps, in_aps)
    nc.compile()
    nc.m = get_hw_module(nc.m)
    _COMPILED = nc
    return nc


def kernel(predicted_locs, predicted_scores, boxes, labels, priors_cxcy):
    """Full (unsharded) inputs -> full scalar output. Data-parallel over N on 8 cores."""
    global LAST_EXEC_NS
    from concourse import bass_utils

    predicted_locs = np.ascontiguousarray(predicted_locs, np.float32)
    predicted_scores = np.ascontiguousarray(predicted_scores, np.float32)
    boxes = np.ascontiguousarray(boxes, np.float32)
    priors_cxcy = np.ascontiguousarray(priors_cxcy, np.float32)

    shared = prep_shared_inputs(priors_cxcy)
    in_maps = []
    for n in range(N_CORES):
        m = dict(shared)
        m.update(prep_core_inputs(predicted_scores[n], predicted_locs[n], boxes[n]))
        in_maps.append(m)

    nc = _build_module()
    trace = _os.environ.get("KERNEL_TRACE", "0") == "1"
    if trace:
        _install_ntff_hook()
    res = bass_utils.run_bass_kernel_spmd(
        nc, in_maps, core_ids=list(range(N_CORES)), trace=trace,
    )
    LAST_EXEC_NS = res.exec_time_ns
    parts = [res.results[n]["part"] for n in range(N_CORES)]
    return combine_partials(parts)


# revision 18
# speedup vs baseline: 1.3398x; 1.0367x over previous
"""MultiBox loss kernel for Trainium2 (Bass/Tile).

Layout: per core, one sample n. Priors padded 8732 -> 8832 = 128*69.
Prior p lives at (partition q = p // 69, column i = p % 69).
Dense tiles are (128, 1380) "i-major": free index i*20 + c.
Box-broadcast tiles are (128, 320) c-major: free index c*16 + m.

Match score: d = ln(inter) - ln(S') with S' = (areaA+areaB)*e^-SIG, so
d = ln(inter/S) + SIG.  iou >= 0.5  <=>  d >= SIG + ln(1/3).
Packing: qm = (d & ~0x7FF) | (15-m); QPA accumulates (qm | 16*(68-i))
max over i.  DVE STT ops (2x_2p mode, 0.5 cyc/el fp32) carry most of
the elementwise work; bitwise ORs of raw-bit codes ride STT scalars
(the STT scalar path preserves arbitrary bit patterns).
"""
import numpy as np

import concourse.bass as bass
import concourse.mybir as mybir
from concourse import tile
from concourse.alu_op_type import AluOpType
from concourse.bass import IndirectOffsetOnAxis

# ---------------- constants ----------------
C, P, M = 20, 8732, 16
QP, I = 128, 69           # partitions x columns
PP = QP * I               # 8832
CM = C * M                # 320
IC = I * C                # 1380
NEG_POS_RATIO = 3.0
SIG = 4.6                 # score shift
ESIG = float(np.exp(-SIG))
_thr = np.float32(np.float32(np.log(np.float32(1.0 / 3.0))) + np.float32(SIG))
THRP = float(np.int32(int(_thr.view(np.int32)) & ~0x7FF).view(np.float32))
SEL_ROWS, SEL_F = 80, 2208   # selection layout: 4 partitions x (69*32) per class
BISECT_ITERS = 9
DUMP_OFF = 10_000_000     # out-of-bounds scatter offset (dropped)
LN_MIN, LN_RANGE = -15.2, 9.3   # range of 5*ln(w) for box sizes

F32 = mybir.dt.float32
I32 = mybir.dt.int32
AF = mybir.ActivationFunctionType
AX = mybir.AxisListType

# ---------------- custom DVE ops ----------------
_REGISTERED = {}


def _register_op(name, spec, subdim=False):
    if name in _REGISTERED:
        return _REGISTERED[name]
    from concourse.dve_ops import DveOp, OPS, CUSTOM_DVE_SPECS, _SUB_OPCODE_FOR_NAME, _CUSTOM_DVE_ROW_BASE
    from concourse.dve_spec import lower, _has_src1
    from concourse.dve_uop import DveOpSpec
    row = _CUSTOM_DVE_ROW_BASE + len(OPS)
    assert row < 0x20
    _SUB_OPCODE_FOR_NAME[name] = row
    shas = {}
    for ver in ("v3", "v4"):
        s = DveOpSpec(name=name, opcode=row, uops=lower(spec, ver=ver), rd1_en=_has_src1(spec))
        shas[ver] = s.sha(ver)
    op = DveOp(name, spec, subdim=subdim, uops_sha=shas)
    OPS.append(op)
    CUSTOM_DVE_SPECS[name] = spec
    _REGISTERED[name] = op
    return op


def get_ops():
    from concourse.dve_spec import (Spec, Src0, Src1, C0, C1, C2, Zero,
                                    maxx, minn, select, AluOp, Idx, Bin)

    ovl = _register_op("ANT_OVL", Spec(
        body=maxx(minn(Src0, C0) - maxx(Src1, C1), C2),
        reference=lambda in0, in1, s0, s1, imm2: np.maximum(
            np.minimum(in0, s0) - np.maximum(in1, s1), imm2).astype(np.float32),
    ))

    def _idxmax_ref(in0, in1, s0, s1, imm2):
        n = in0.shape[1]
        out = np.where(in0 >= s0, s1 - np.arange(n)[None, :], 0.0).astype(np.float32)
        return out, out.max(axis=1, keepdims=True)

    idxmax = _register_op("ANT_IDXMAX", Spec(
        body=select(Src0 >= C0, C1 - Idx, Zero),
        accum=AluOp.MAX,
        reference=_idxmax_ref,
    ))

    def _selmax_ref(in0, in1, s0, s1, imm2):
        out = np.where(in0 >= s0, in1, 0.0).astype(np.float32)
        return out, out.max(axis=1, keepdims=True)

    selmax = _register_op("ANT_SELMAX", Spec(
        body=select(Src0 >= C0, Src1, Zero),
        accum=AluOp.MAX,
        reference=_selmax_ref,
    ))

    absd = _register_op("ANT_ABSD", Spec(
        body=Bin(AluOp.ABSOLUTE_DIFF, Src0, Src1),
        reference=lambda in0, in1, s0, s1, imm2: np.abs(in0 - in1).astype(np.float32),
    ))

    absds = _register_op("ANT_ABSDS", Spec(
        body=Bin(AluOp.ABSOLUTE_DIFF, Src0, Src1 * C0),
        reference=lambda in0, in1, s0, s1, imm2: np.abs(in0 - in1 * s0).astype(np.float32),
    ))

    def _sumgt_ref(in0, in1, s0, s1, imm2):
        out = np.where(in0 > s0, in0, 0.0).astype(np.float32)
        return out, out.sum(axis=1, keepdims=True, dtype=np.float32)

    sumgt = _register_op("ANT_SUMGT", Spec(
        body=select(Src0 > C0, Src0, Zero),
        accum=AluOp.ADD,
        reference=_sumgt_ref,
    ))
    return ovl, idxmax, selmax, sumgt, absd, absds


# ---------------- host-side input prep ----------------
def prep_core_inputs(scores_nc, locs_nc, boxes_nc):
    sc = np.zeros((C, QP * 138), np.float32)
    sc[:, : P * 2] = scores_nc.reshape(C, P * 2)
    lc = np.zeros((C, QP * 276), np.float32)
    lc[:, : P * 4] = locs_nc.reshape(C, P * 4)
    return {
        "scores_pad": sc,
        "locs_pad": lc,
        "boxes_t": boxes_nc.reshape(1, CM * 4).astype(np.float32),
    }


def prep_shared_inputs(priors):
    pr = np.zeros((PP, 4), np.float32)
    pr[:P] = priors
    pr[P:, 0] = 50.0 + np.arange(PP - P)
    pr[P:, 1] = 50.0
    pr[P:, 2] = 0.01
    pr[P:, 3] = 0.01

    ident = np.eye(QP, dtype=np.float32)
    ind120 = np.zeros((SEL_ROWS, C), np.float32)
    for k in range(SEL_ROWS):
        ind120[k, k // 4] = 1.0
    indT = np.ascontiguousarray(ind120.T)
    later = np.zeros((QP, QP), np.float32)
    for a in range(QP):
        for b in range(QP):
            if b > a and b // M == a // M:
                later[a, b] = 1.0
    pidx = np.arange(QP)[:, None] * I + np.arange(I)[None, :]   # (128, 69)
    padmask = (pidx < P).astype(np.float32)[:, :, None].repeat(C, 2).reshape(QP, IC)
    part = np.arange(QP)
    coffs = np.stack([((b * QP + part) // M).astype(np.float32) for b in range(3)], 1)
    mvals = np.stack([(15.0 - (b * QP + part) % M).astype(np.float32) for b in range(3)], 1)
    # per-column packed codes: (15-m) | 16*(68-i), partition-invariant
    iidx = np.arange(I)
    cm_m = np.arange(CM) % M
    codes = ((16 * (68 - iidx))[:, None] | (15 - cm_m)[None, :]).astype(np.int32)  # (I, CM)
    mdcol = np.broadcast_to(codes.reshape(1, I * CM), (QP, I * CM))
    mdcol = np.ascontiguousarray(mdcol).view(np.float32)
    return {
        "mdcol": mdcol,
        "priors_t": pr,
        "ident": ident,
        "ind120": ind120,
        "indT": indT,
        "later": later,
        "coffs": coffs,
        "mvals": mvals,
        "padmask": padmask,
    }


# ---------------- the kernel ----------------
def build_kernel(tc, outs, ins):
    nc = tc.nc
    OVL, IDXMAX, SELMAX, SUMGT, ABSD, ABSDS = get_ops()

    out_part = outs["part"]      # (8, 20) f32

    from contextlib import ExitStack
    with ExitStack() as ctx:
        cpool = ctx.enter_context(tc.tile_pool(name="const", bufs=1))
        lpool = ctx.enter_context(tc.tile_pool(name="loop", bufs=2))
        ppool = ctx.enter_context(tc.tile_pool(name="psum", bufs=2, space="PSUM"))
        dpool = ctx.enter_context(tc.tile_pool(name="dram", bufs=1, space="DRAM"))
        _build(nc, tc, cpool, lpool, ppool, dpool, ins, out_part,
               OVL, IDXMAX, SELMAX, SUMGT, ABSD, ABSDS)


def _build(nc, tc, cpool, lpool, ppool, dpool, ins, out_part, OVL, IDXMAX, SELMAX, SUMGT, ABSD, ABSDS):
    scores = ins["scores_pad"]
    locs = ins["locs_pad"]
    boxes_t = ins["boxes_t"]
    priors_t = ins["priors_t"]
    stt = nc.vector.scalar_tensor_tensor

    # ---- load constants / inputs ----
    PR = cpool.tile([QP, I, 4], F32)
    nc.sync.dma_start(out=PR[:], in_=priors_t.rearrange("(q i) k -> q i k", q=QP))
    IDENT = cpool.tile([QP, QP], F32)
    nc.sync.dma_start(out=IDENT[:], in_=ins["ident"])
    IND120 = cpool.tile([SEL_ROWS, C], F32)
    nc.sync.dma_start(out=IND120[:], in_=ins["ind120"])
    INDT = cpool.tile([C, SEL_ROWS], F32)
    nc.sync.dma_start(out=INDT[:], in_=ins["indT"])
    LATER = cpool.tile([QP, QP], F32)
    nc.sync.dma_start(out=LATER[:], in_=ins["later"])

    BT = cpool.tile([1, CM * 4], F32)
    nc.sync.dma_start(out=BT[:], in_=boxes_t)
    SC = cpool.tile([QP, C, 138], F32)
    nc.sync.dma_start(out=SC[:], in_=scores.rearrange("c (q e) -> q c e", q=QP))

    PADM = cpool.tile([QP, IC], F32, tag="padm")
    nc.sync.dma_start(out=PADM[:], in_=ins["padmask"])
    CONSTI = cpool.tile([QP, 9], I32)
    # 0: pack mask ~0x7FF, 1: col extract 0x7F0, 2: m extract 0xF,
    # 3: 63, 4: 0xFC0, 5: 0x3F000, 6: 0xFC0000, 7: unused, 8: 0
    for _k, _v in enumerate([~0x7FF, 0x7F0, 0xF, 63, 0xFC0, 0x3F000, 0xFC0000, 0, 0]):
        nc.vector.memset(CONSTI[:, _k:_k + 1], _v)
    COFF = cpool.tile([QP, 3], F32)
    nc.sync.dma_start(out=COFF[:], in_=ins["coffs"])
    VALS = cpool.tile([QP, 3], F32)
    nc.sync.dma_start(out=VALS[:], in_=ins["mvals"])

    # ---- prior-derived tiles (128, 69) ----
    pcx = PR[:, :, 0]
    pcy = PR[:, :, 1]
    pw = PR[:, :, 2]
    ph = PR[:, :, 3]
    PX1 = cpool.tile([QP, I], F32)
    PX2 = cpool.tile([QP, I], F32)
    PY1 = cpool.tile([QP, I], F32)
    PY2 = cpool.tile([QP, I], F32)
    PAREA = cpool.tile([QP, I], F32)   # prior area pre-scaled by exp(-SIG)
    stt(out=PX1[:], in0=pw, scalar=-0.5, in1=pcx, op0=AluOpType.mult, op1=AluOpType.add)
    stt(out=PX2[:], in0=pw, scalar=0.5, in1=pcx, op0=AluOpType.mult, op1=AluOpType.add)
    stt(out=PY1[:], in0=ph, scalar=-0.5, in1=pcy, op0=AluOpType.mult, op1=AluOpType.add)
    stt(out=PY2[:], in0=ph, scalar=0.5, in1=pcy, op0=AluOpType.mult, op1=AluOpType.add)
    stt(out=PAREA[:], in0=pw, scalar=ESIG, in1=ph, op0=AluOpType.mult, op1=AluOpType.mult)
    IPW = cpool.tile([QP, I], F32)   # 10 / pw
    IPH = cpool.tile([QP, I], F32)
    scr69 = cpool.tile([QP, I], F32)
    nc.vector.reciprocal_approx_accurate(out=IPW[:], in_=pw, scratch=scr69[:])
    nc.vector.reciprocal_approx_accurate(out=IPH[:], in_=ph, scratch=scr69[:])
    nc.vector.tensor_scalar_mul(IPW[:], IPW[:], 10.0)
    nc.vector.tensor_scalar_mul(IPH[:], IPH[:], 10.0)
    LPW5 = cpool.tile([QP, I], F32)  # 5*ln(pw)
    LPH5 = cpool.tile([QP, I], F32)
    nc.scalar.activation(out=LPW5[:], in_=pw, func=AF.Ln)
    nc.scalar.activation(out=LPH5[:], in_=ph, func=AF.Ln)
    nc.vector.tensor_scalar_mul(LPW5[:], LPW5[:], 5.0)
    nc.vector.tensor_scalar_mul(LPH5[:], LPH5[:], 5.0)

    # per-prior l1 helper tiles (no loop deps; emitted early for overlap)
    IPW63 = cpool.tile([QP, I], F32)
    IPH63 = cpool.tile([QP, I], F32)
    nc.vector.tensor_scalar_mul(IPW63[:], IPW[:], 1.0 / 63.0)
    nc.vector.tensor_scalar_mul(IPH63[:], IPH[:], 1.0 / (63.0 * 64.0))
    PCXI = cpool.tile([QP, I], F32)
    PCYI = cpool.tile([QP, I], F32)
    nc.vector.tensor_tensor(out=PCXI[:], in0=pcx, in1=IPW[:], op=AluOpType.mult)
    nc.vector.tensor_tensor(out=PCYI[:], in0=pcy, in1=IPH[:], op=AluOpType.mult)

    # ---- box-derived broadcast tiles (128, 320) c-major ----
    bx1v = BT[:, 0::4]
    by1v = BT[:, 1::4]
    bx2v = BT[:, 2::4]
    by2v = BT[:, 3::4]
    BD = cpool.tile([1, CM * 10], F32, tag="cbslot")
    s = [BD[:, k * CM:(k + 1) * CM] for k in range(10)]
    # 0 bx1, 1 bx2, 2 by1, 3 by2, 4 bcx, 5 bcy, 6 areab*e^-SIG, 7 lnbw5, 8 lnbh5, 9 quad
    nc.vector.tensor_copy(out=s[0], in_=bx1v)
    nc.vector.tensor_copy(out=s[1], in_=bx2v)
    nc.vector.tensor_copy(out=s[2], in_=by1v)
    nc.vector.tensor_copy(out=s[3], in_=by2v)
    t1 = cpool.tile([1, CM], F32)
    nc.vector.tensor_tensor(out=t1[:], in0=bx1v, in1=bx2v, op=AluOpType.add)
    nc.vector.tensor_scalar_mul(s[4], t1[:], 0.5)
    nc.vector.tensor_tensor(out=t1[:], in0=by1v, in1=by2v, op=AluOpType.add)
    nc.vector.tensor_scalar_mul(s[5], t1[:], 0.5)
    tbw = cpool.tile([1, CM], F32)
    tbh = cpool.tile([1, CM], F32)
    nc.vector.tensor_tensor(out=tbw[:], in0=bx2v, in1=bx1v, op=AluOpType.subtract)
    nc.vector.tensor_tensor(out=tbh[:], in0=by2v, in1=by1v, op=AluOpType.subtract)
    stt(out=s[6], in0=tbw[:], scalar=ESIG, in1=tbh[:], op0=AluOpType.mult, op1=AluOpType.mult)
    nc.scalar.activation(out=s[7], in_=tbw[:], func=AF.Ln)
    nc.scalar.activation(out=s[8], in_=tbh[:], func=AF.Ln)
    nc.vector.tensor_scalar_mul(s[7], s[7], 5.0)
    nc.vector.tensor_scalar_mul(s[8], s[8], 5.0)
    # quad code: round(bcx*63) + 64*round(bcy*63) + 64^2*round((lnw5-LN_MIN)*63/LN_RANGE) + 64^3*(...)
    enc = cpool.tile([1, CM], I32)
    encf = cpool.tile([1, CM], F32)
    quad = s[9]
    nc.vector.tensor_scalar(out=enc[:], in0=s[4], scalar1=63.0, scalar2=0.5,
                            op0=AluOpType.mult, op1=AluOpType.add)
    nc.vector.tensor_copy(out=quad, in_=enc[:])
    nc.vector.tensor_scalar(out=enc[:], in0=s[5], scalar1=63.0, scalar2=0.5,
                            op0=AluOpType.mult, op1=AluOpType.add)
    nc.vector.tensor_copy(out=encf[:], in_=enc[:])
    stt(out=quad, in0=encf[:], scalar=64.0, in1=quad, op0=AluOpType.mult, op1=AluOpType.add)
    nc.vector.tensor_scalar(out=enc[:], in0=s[7], scalar1=63.0 / LN_RANGE,
                            scalar2=-LN_MIN * 63.0 / LN_RANGE + 0.5,
                            op0=AluOpType.mult, op1=AluOpType.add)
    nc.vector.tensor_copy(out=encf[:], in_=enc[:])
    stt(out=quad, in0=encf[:], scalar=4096.0, in1=quad, op0=AluOpType.mult, op1=AluOpType.add)
    nc.vector.tensor_scalar(out=enc[:], in0=s[8], scalar1=63.0 / LN_RANGE,
                            scalar2=-LN_MIN * 63.0 / LN_RANGE + 0.5,
                            op0=AluOpType.mult, op1=AluOpType.add)
    nc.vector.tensor_copy(out=encf[:], in_=enc[:])
    stt(out=quad, in0=encf[:], scalar=262144.0, in1=quad, op0=AluOpType.mult, op1=AluOpType.add)

    ONES1 = cpool.tile([1, QP], F32)
    nc.vector.memset(ONES1[:], 1.0)
    BB = cpool.tile([QP, CM * 10], F32)
    tot = CM * 10
    off = 0
    while off < tot:
        w = min(512, tot - off)
        pt = ppool.tile([QP, w], F32, tag="bcast")
        nc.tensor.matmul(out=pt[:], lhsT=ONES1[:], rhs=BD[:, off:off + w], start=True, stop=True)
        nc.scalar.copy(out=BB[:, off:off + w], in_=pt[:])
        off += w
    BX1 = BB[:, 0 * CM:1 * CM]
    BX2 = BB[:, 1 * CM:2 * CM]
    BY1 = BB[:, 2 * CM:3 * CM]
    BY2 = BB[:, 3 * CM:4 * CM]
    BAR = BB[:, 6 * CM:7 * CM]
    QUADB = BB[:, 9 * CM:10 * CM]

    PL = cpool.tile([QP, C, 276], F32)
    nc.sync.dma_start(out=PL[:], in_=locs.rearrange("c (q e) -> q c e", q=QP))

    # ---- CE (no dependency on matching; emitted early for engine overlap) ----
    DM = cpool.tile([QP, IC], F32, tag="dm")
    sc4 = SC[:].rearrange("p c (i two) -> p c i two", two=2)
    dm3 = DM[:].rearrange("p (i c) -> p i c", c=C)
    stt(out=dm3.rearrange("p i c -> p c i"), in0=sc4[:, :, :, 1], scalar=1.0,
        in1=sc4[:, :, :, 0], op0=AluOpType.mult, op1=AluOpType.subtract)
    CE = cpool.tile([QP, IC], F32)
    nc.scalar.activation(out=CE[:], in_=DM[:], func=AF.Exp)
    nc.scalar.activation(out=CE[:], in_=CE[:], func=AF.Ln, bias=1.0)

    NEG1 = cpool.tile([QP, IC], F32, tag="l1a")
    nc.vector.memset(NEG1[:], -1.0)

    # ---- accumulators ----
    QMM = cpool.tile([QP, I, C], F32)
    QPA = cpool.tile([QP, CM], F32)
    nc.vector.memset(QPA[:], 0.0)
    mdcol_d = ins["mdcol"]

    # ================= main loop over columns i =================
    MCH = 4   # mdcol prefetch chunk
    for i in range(I):
        if i % MCH == 0:
            nch = min(MCH, I - i)
            MDCOL = lpool.tile([QP, MCH * CM], F32, tag="mdcol")
            nc.sync.dma_start(out=MDCOL[:, :nch * CM],
                              in_=mdcol_d[:, i * CM:(i + nch) * CM])
        xov = lpool.tile([QP, CM], F32, tag="xov")
        nc.vector._custom_dve(OVL, out=xov[:], in0=BX2, in1=BX1,
                              s0=PX2[:, i:i + 1], s1=PX1[:, i:i + 1], imm2=1e-18)
        yov = lpool.tile([QP, CM], F32, tag="yov")
        nc.vector._custom_dve(OVL, out=yov[:], in0=BY2, in1=BY1,
                              s0=PY2[:, i:i + 1], s1=PY1[:, i:i + 1], imm2=1e-18)
        inter = lpool.tile([QP, CM], F32, tag="inter")
        stt(out=inter[:], in0=xov[:], scalar=1.0, in1=yov[:],
            op0=AluOpType.mult, op1=AluOpType.mult)
        S = lpool.tile([QP, CM], F32, tag="S")
        nc.scalar.activation(out=S[:], in_=BAR, func=AF.Identity,
                             bias=PAREA[:, i:i + 1], scale=1.0)
        lnI = lpool.tile([QP, CM], F32, tag="lnI")
        nc.scalar.activation(out=lnI[:], in_=inter[:], func=AF.Ln)
        lnS = lpool.tile([QP, CM], F32, tag="lnS")
        nc.scalar.activation(out=lnS[:], in_=S[:], func=AF.Ln)
        d = lpool.tile([QP, CM], F32, tag="d")
        stt(out=d[:], in0=lnI[:], scalar=1.0, in1=lnS[:],
            op0=AluOpType.mult, op1=AluOpType.subtract)
        if i % 2 == 0:
            QM2 = lpool.tile([QP, 2 * CM], F32, tag="qm2")
        qmv = QM2[:, (i % 2) * CM:(i % 2 + 1) * CM]
        stt(out=qmv.bitcast(I32), in0=d[:].bitcast(I32), scalar=CONSTI[:, 0:1],
            in1=MDCOL[:, (i % MCH) * CM:(i % MCH + 1) * CM].bitcast(I32),
            op0=AluOpType.bitwise_and, op1=AluOpType.bitwise_or)
        stt(out=QPA[:], in0=qmv, scalar=1.0, in1=QPA[:],
            op0=AluOpType.mult, op1=AluOpType.max)
        if i % 2 == 1:
            nc.vector.tensor_reduce(
                out=QMM[:, i - 1:i + 1, :],
                in_=QM2[:].rearrange("p (x m) -> p x m", m=M),
                axis=AX.X, op=AluOpType.max)
        elif i == I - 1:
            nc.vector.tensor_reduce(
                out=QMM[:, i, :],
                in_=qmv.rearrange("p (c m) -> p c m", m=M),
                axis=AX.X, op=AluOpType.max)

    # FMD scratch init (DMA drains during the loop; only needed at scatter time)
    FMD = dpool.tile([PP * C, 1], F32)
    nc.sync.dma_start(out=FMD[:].rearrange("(q f) one -> q (f one)", q=QP), in_=NEG1[:])

    QMMf = QMM[:].rearrange("p i c -> p (i c)")
    QMMi = QMMf.bitcast(I32)

    # ================= pos mask, m* =================
    POSB = cpool.tile([QP, IC], F32, tag="posb")
    nc.vector.tensor_scalar(out=POSB[:], in0=QMMf, scalar1=THRP, scalar2=0.0,
                            op0=AluOpType.is_ge, op1=AluOpType.max)
    # m-code (15-m) in low 4 bits
    MSI = cpool.tile([QP, IC], I32, tag="ic_int")
    stt(out=MSI[:], in0=QMMi, scalar=CONSTI[:, 2:3],
        in1=CONSTI[:, 8:9].to_broadcast([QP, IC]),
        op0=AluOpType.bitwise_and, op1=AluOpType.bitwise_or)
    MS = cpool.tile([QP, IC], F32)
    nc.vector.tensor_copy(out=MS[:], in_=MSI[:])

    # ================= prior_for_obj (forced positives) =================
    QPAf = QPA[:]
    PSTARI = cpool.tile([QP, 3], I32)
    for b in range(3):
        w = 128 if b < 2 else 64
        tp = ppool.tile([QP, QP], F32, tag="ptr")
        nc.tensor.transpose(out=tp[:w, :], in_=QPAf[:, b * QP:b * QP + w], identity=IDENT[:])
        TQ = lpool.tile([QP, QP], F32, tag="TQ")
        nc.scalar.copy(out=TQ[:w, :], in_=tp[:w, :])
        vmax = lpool.tile([QP, 1], F32, tag="vmax")
        nc.vector.tensor_reduce(out=vmax[:w], in_=TQ[:w, :], axis=AX.X, op=AluOpType.max)
        qd = lpool.tile([QP, 1], F32, tag="qd")
        sc1 = lpool.tile([QP, QP], F32, tag="sc1")
        nc.vector._custom_dve(IDXMAX, out=sc1[:w, :], accum_out=qd[:w], in0=TQ[:w, :],
                              s0=vmax[:w], s1=127.0)
        TLI = lpool.tile([QP, QP], I32, tag="TLI")
        stt(out=TLI[:w, :], in0=TQ[:w, :].bitcast(I32), scalar=CONSTI[:w, 1:2],
            in1=CONSTI[:w, 8:9].to_broadcast([w, QP]),
            op0=AluOpType.bitwise_and, op1=AluOpType.bitwise_or)
        TLF = lpool.tile([QP, QP], F32, tag="TLF")
        nc.vector.tensor_copy(out=TLF[:w, :], in_=TLI[:w, :])
        colv = lpool.tile([QP, 1], F32, tag="ilow")
        sc2 = lpool.tile([QP, QP], F32, tag="sc2")
        nc.vector._custom_dve(SELMAX, out=sc2[:w, :], accum_out=colv[:w], in0=TQ[:w, :],
                              in1=TLF[:w, :], s0=vmax[:w])
        # p* = (127 - qd)*69 + (68 - colv/16)
        pst = lpool.tile([QP, 1], F32, tag="pst")
        nc.vector.tensor_scalar(out=pst[:w], in0=qd[:w], scalar1=-69.0,
                                scalar2=float(127 * 69 + 68),
                                op0=AluOpType.mult, op1=AluOpType.add)
        stt(out=pst[:w], in0=colv[:w], scalar=-1.0 / 16.0, in1=pst[:w],
            op0=AluOpType.mult, op1=AluOpType.add)
        # dedup: later m with same p* in same class wins
        tpp = ppool.tile([QP, QP], F32, tag="ptr")
        nc.tensor.transpose(out=tpp[:, :w], in_=pst[:w, :1].to_broadcast([w, QP]),
                            identity=IDENT[:w, :w])
        PTT = lpool.tile([QP, QP], F32, tag="PTT")
        nc.scalar.copy(out=PTT[:, :w], in_=tpp[:, :w])
        EQM = lpool.tile([QP, QP], F32, tag="EQM")
        nc.vector.tensor_tensor(out=EQM[:w, :w], in0=pst[:w, :1].to_broadcast([w, w]),
                                in1=PTT[:w, :w], op=AluOpType.is_equal)
        nc.vector.tensor_tensor(out=EQM[:w, :w], in0=EQM[:w, :w], in1=LATER[:w, :w],
                                op=AluOpType.mult)
        dom = lpool.tile([QP, 1], F32, tag="dom")
        nc.vector.tensor_reduce(out=dom[:w], in_=EQM[:w, :w], axis=AX.X, op=AluOpType.max)
        # offset = p* * 20 + c; dominated -> +DUMP_OFF (dropped by bounds check)
        offf = lpool.tile([QP, 1], F32, tag="offf")
        stt(out=offf[:w], in0=pst[:w], scalar=20.0, in1=COFF[:w, b:b + 1],
            op0=AluOpType.mult, op1=AluOpType.add)
        stt(out=offf[:w], in0=dom[:w], scalar=float(DUMP_OFF), in1=offf[:w],
            op0=AluOpType.mult, op1=AluOpType.add)
        nc.vector.tensor_copy(out=PSTARI[:w, b:b + 1], in_=offf[:w])

    for b in range(3):
        w = 128 if b < 2 else 64
        nc.gpsimd.indirect_dma_start(
            out=FMD[:],
            out_offset=IndirectOffsetOnAxis(ap=PSTARI[:w, b:b + 1], axis=0),
            in_=VALS[:w, b:b + 1],
            in_offset=None,
            bounds_check=PP * C - 1,
            oob_is_err=False,
        )
    FM = cpool.tile([QP, IC], F32, tag="fm")
    nc.sync.dma_start(out=FM[:], in_=FMD[:].rearrange("(q f) one -> q (f one)", q=QP))

    FGE = cpool.tile([QP, IC], F32)
    nc.vector.tensor_scalar(out=FGE[:], in0=FM[:], scalar1=0.0, scalar2=0.0,
                            op0=AluOpType.is_ge, op1=AluOpType.max)
    POSB2 = POSB
    nc.vector.tensor_tensor(out=POSB2[:], in0=POSB[:], in1=FGE[:], op=AluOpType.max)
    FGEI = cpool.tile([QP, IC], I32, tag="ic_int")
    nc.vector.tensor_copy(out=FGEI[:], in_=FGE[:])
    MS2 = MS
    nc.vector.copy_predicated(out=MS2[:], mask=FGEI[:], data=FM[:])

    # ================= CE pos/neg splits =================
    CEP = cpool.tile([QP, IC], F32, tag="cep")
    stt(out=CEP[:], in0=PADM[:], scalar=1.0, in1=POSB2[:],
        op0=AluOpType.mult, op1=AluOpType.subtract)
    CEN = cpool.tile([QP, C, I], F32, tag="scbslot")
    cen_im = CEN[:].rearrange("p c i -> p i c")
    stt(out=cen_im, in0=CE[:].rearrange("p (i c) -> p i c", c=C), scalar=1.0,
        in1=CEP[:].rearrange("p (i c) -> p i c", c=C),
        op0=AluOpType.mult, op1=AluOpType.mult)
    CPT = cpool.tile([QP, IC], F32, tag="gt")
    stt(out=CPT[:], in0=CE[:], scalar=1.0, in1=DM[:],
        op0=AluOpType.mult, op1=AluOpType.subtract)
    stt(out=CPT[:], in0=CPT[:], scalar=1.0, in1=POSB2[:],
        op0=AluOpType.mult, op1=AluOpType.mult)

    # ================= counts / class sums =================
    NPQ = cpool.tile([QP, C], F32)
    nc.vector.tensor_reduce(out=NPQ[:], in_=POSB2[:].rearrange("p (i c) -> p c i", c=C),
                            axis=AX.X, op=AluOpType.add)
    CPQ = cpool.tile([QP, C], F32)
    nc.vector.tensor_reduce(out=CPQ[:], in_=CPT[:].rearrange("p (i c) -> p c i", c=C),
                            axis=AX.X, op=AluOpType.add)
    ONESC = cpool.tile([QP, 1], F32)
    nc.vector.memset(ONESC[:], 1.0)
    NPC_p = ppool.tile([1, C], F32, tag="pmm")
    nc.tensor.matmul(out=NPC_p[:], lhsT=ONESC[:], rhs=NPQ[:], start=True, stop=True)
    CPC_p = ppool.tile([1, C], F32, tag="pmm")
    nc.tensor.matmul(out=CPC_p[:], lhsT=ONESC[:], rhs=CPQ[:], start=True, stop=True)
    NPC = cpool.tile([1, C], F32)
    nc.scalar.copy(out=NPC[:], in_=NPC_p[:])
    CPC = cpool.tile([1, C], F32)
    nc.scalar.copy(out=CPC[:], in_=CPC_p[:])

    kp = ppool.tile([C, 1], F32, tag="pmm")
    nc.tensor.transpose(out=kp[:], in_=NPC[:], identity=IDENT[:1, :1])
    KC = cpool.tile([C, 1], F32)
    nc.scalar.copy(out=KC[:], in_=kp[:])
    nc.vector.tensor_scalar_mul(KC[:], KC[:], NEG_POS_RATIO)

    # ================= hard-negative selection =================
    CB = cpool.tile([SEL_ROWS, SEL_F], F32, tag="cbslot")
    for c in range(C):
        nc.sync.dma_start(out=CB[c * 4:(c + 1) * 4, :], in_=CEN[:, c, :])

    LO = cpool.tile([C, 1], F32)
    HI = cpool.tile([C, 1], F32)
    TC_ = cpool.tile([C, 1], F32)
    nc.vector.memset(LO[:], 0.8)
    nc.vector.memset(HI[:], 4.0)
    T120 = cpool.tile([SEL_ROWS, 1], F32)
    CNT6 = cpool.tile([SEL_ROWS, 1], F32)
    CNTC = cpool.tile([C, 1], F32)
    scb = cpool.tile([SEL_ROWS, SEL_F], F32, tag="scbslot")
    for it in range(BISECT_ITERS):
        nc.vector.tensor_tensor(out=TC_[:], in0=LO[:], in1=HI[:], op=AluOpType.add)
        nc.vector.tensor_scalar_mul(TC_[:], TC_[:], 0.5)
        tp120 = ppool.tile([SEL_ROWS, 1], F32, tag="pmm")
        nc.tensor.matmul(out=tp120[:], lhsT=INDT[:], rhs=TC_[:], start=True, stop=True)
        nc.scalar.copy(out=T120[:], in_=tp120[:])
        nc.vector.tensor_scalar(out=scb[:], in0=CB[:], scalar1=T120[:, :1], scalar2=0.0,
                                op0=AluOpType.is_gt, op1=AluOpType.add, accum_out=CNT6[:])
        tpc = ppool.tile([C, 1], F32, tag="pmm")
        nc.tensor.matmul(out=tpc[:], lhsT=IND120[:], rhs=CNT6[:], start=True, stop=True)
        nc.scalar.copy(out=CNTC[:], in_=tpc[:])
        gm = lpool.tile([C, 1], I32, tag="gm")
        nc.vector.tensor_tensor(out=gm[:], in0=CNTC[:], in1=KC[:], op=AluOpType.is_ge)
        nc.vector.copy_predicated(out=LO[:], mask=gm[:], data=TC_[:])
        lm = lpool.tile([C, 1], I32, tag="lm")
        nc.vector.tensor_tensor(out=lm[:], in0=CNTC[:], in1=KC[:], op=AluOpType.is_lt)
        nc.vector.copy_predicated(out=HI[:], mask=lm[:], data=TC_[:])
    tp120 = ppool.tile([SEL_ROWS, 1], F32, tag="pmm")
    nc.tensor.matmul(out=tp120[:], lhsT=INDT[:], rhs=LO[:], start=True, stop=True)
    nc.scalar.copy(out=T120[:], in_=tp120[:])
    SUM6 = cpool.tile([SEL_ROWS, 1], F32)
    nc.vector._custom_dve(SUMGT, out=scb[:], accum_out=SUM6[:], in0=CB[:], s0=T120[:, :1])
    nc.vector.tensor_scalar(out=scb[:], in0=CB[:], scalar1=T120[:, :1], scalar2=0.0,
                            op0=AluOpType.is_gt, op1=AluOpType.add, accum_out=CNT6[:])
    SUMC_p = ppool.tile([C, 1], F32, tag="pmm")
    nc.tensor.matmul(out=SUMC_p[:], lhsT=IND120[:], rhs=SUM6[:], start=True, stop=True)
    CNTC_p = ppool.tile([C, 1], F32, tag="pmm")
    nc.tensor.matmul(out=CNTC_p[:], lhsT=IND120[:], rhs=CNT6[:], start=True, stop=True)
    CH = cpool.tile([C, 1], F32)
    nc.scalar.copy(out=CNTC[:], in_=CNTC_p[:])
    nc.vector.tensor_tensor(out=CH[:], in0=KC[:], in1=CNTC[:], op=AluOpType.subtract)
    nc.vector.tensor_tensor(out=CH[:], in0=CH[:], in1=LO[:], op=AluOpType.mult)
    SUMC = cpool.tile([C, 1], F32)
    nc.scalar.copy(out=SUMC[:], in_=SUMC_p[:])
    nc.vector.tensor_tensor(out=CH[:], in0=CH[:], in1=SUMC[:], op=AluOpType.add)

    # ================= localization loss =================
    # 16-way select of quad-encoded box quantities by m* code (15-m);
    # selects are disjoint so plain adds accumulate G.
    G = cpool.tile([QP, IC], F32, tag="gt")
    g3 = G[:].rearrange("p (i c) -> p i c", c=C)
    QUADC = cpool.tile([QP, CM], F32)   # m-major reorder for contiguous in1
    nc.vector.tensor_copy(out=QUADC[:].rearrange("p (m c) -> p m c", m=M),
                          in_=QUADB.rearrange("p (c m) -> p m c", m=M))

    def quadview(m):
        return QUADC[:, m * C:(m + 1) * C].unsqueeze(1).to_broadcast([QP, I, C])

    nc.vector.memset(G[:], 0.0)
    ms3 = MS2[:].rearrange("p (i c) -> p i c", c=C)
    TQM = cpool.tile([QP, I, C], F32, tag="tqm")
    tq3 = TQM[:]
    for m in range(M):
        stt(out=tq3, in0=ms3, scalar=float(15 - m), in1=quadview(m),
            op0=AluOpType.is_equal, op1=AluOpType.mult)
        stt(out=g3, in0=tq3, scalar=1.0, in1=g3, op0=AluOpType.mult, op1=AluOpType.add)
    GI = cpool.tile([QP, IC], I32, tag="ic_int")
    nc.vector.tensor_copy(out=GI[:], in_=G[:])

    L1A = cpool.tile([QP, IC], F32, tag="l1a")
    nc.vector.memset(L1A[:], 0.0)
    EC = cpool.tile([QP, IC], F32, tag="dm")
    ECI = cpool.tile([QP, IC], I32, tag="ec_int")
    TM2 = cpool.tile([QP, IC], F32, tag="cep")
    tm3 = TM2[:].rearrange("p (i c) -> p i c", c=C)
    ec3 = EC[:].rearrange("p (i c) -> p i c", c=C)
    pl5 = PL[:].rearrange("p c (i four) -> p c i four", four=4)

    def bc69(t):
        return t[:].unsqueeze(2).to_broadcast([QP, I, C])

    def l1_xy(mask_col, scale_t, pci_t, k_coord):
        stt(out=ECI[:], in0=GI[:], scalar=CONSTI[:, mask_col:mask_col + 1],
            in1=CONSTI[:, 8:9].to_broadcast([QP, IC]),
            op0=AluOpType.bitwise_and, op1=AluOpType.bitwise_or)
        nc.vector.tensor_copy(out=EC[:], in_=ECI[:])
        # A = pl + pcx*ipw ; t = e * (ipw/63/shift); diff = A - t
        plv = pl5[:, :, :, k_coord].rearrange("p c i -> p i c")
        stt(out=tm3, in0=plv, scalar=1.0, in1=bc69(pci_t),
            op0=AluOpType.mult, op1=AluOpType.add)
        stt(out=ec3, in0=ec3, scalar=1.0, in1=bc69(scale_t),
            op0=AluOpType.mult, op1=AluOpType.mult)
        nc.vector._custom_dve(ABSD, out=TM2[:], in0=TM2[:], in1=EC[:])
        stt(out=L1A[:], in0=TM2[:], scalar=1.0, in1=L1A[:],
            op0=AluOpType.mult, op1=AluOpType.add)

    l1_xy(3, IPW63, PCXI, 0)          # cx: e in [0,63], value e/63 * ipw
    l1_xy(4, IPH63, PCYI, 1)          # cy: e-bits at <<6; scale = iph/(63*64)

    # w/h coords: A = pl + lnpw5 - LN_MIN ; t = e * (LN_RANGE/63/shift)
    def l1_wh(mask_col, shift, lp5, k_coord):
        stt(out=ECI[:], in0=GI[:], scalar=CONSTI[:, mask_col:mask_col + 1],
            in1=CONSTI[:, 8:9].to_broadcast([QP, IC]),
            op0=AluOpType.bitwise_and, op1=AluOpType.bitwise_or)
        nc.vector.tensor_copy(out=EC[:], in_=ECI[:])
        plv = pl5[:, :, :, k_coord].rearrange("p c i -> p i c")
        stt(out=tm3, in0=plv, scalar=-LN_MIN, in1=bc69(lp5),
            op0=AluOpType.add, op1=AluOpType.add)
        nc.vector._custom_dve(ABSDS, out=TM2[:], in0=TM2[:], in1=EC[:],
                              s0=LN_RANGE / 63.0 / shift)
        stt(out=L1A[:], in0=TM2[:], scalar=1.0, in1=L1A[:],
            op0=AluOpType.mult, op1=AluOpType.add)

    l1_wh(5, 4096.0, LPW5, 2)
    l1_wh(6, 262144.0, LPH5, 3)

    stt(out=L1A[:], in0=L1A[:], scalar=1.0, in1=POSB2[:],
        op0=AluOpType.mult, op1=AluOpType.mult)
    L1Q = cpool.tile([QP, C], F32)
    nc.vector.tensor_reduce(out=L1Q[:], in_=L1A[:].rearrange("p (i c) -> p c i", c=C),
                            axis=AX.X, op=AluOpType.add)
    L1C_p = ppool.tile([1, C], F32, tag="pmm")
    nc.tensor.matmul(out=L1C_p[:], lhsT=ONESC[:], rhs=L1Q[:], start=True, stop=True)
    L1C = cpool.tile([1, C], F32)
    nc.scalar.copy(out=L1C[:], in_=L1C_p[:])

    # ================= outputs =================
    chp = ppool.tile([1, C], F32, tag="pmm")
    nc.tensor.transpose(out=chp[:], in_=CH[:, :1], identity=IDENT[:C, :C])
    CHR = cpool.tile([1, C], F32)
    nc.scalar.copy(out=CHR[:], in_=chp[:])
    nc.sync.dma_start(out=out_part[0:1, :], in_=NPC[:])
    nc.sync.dma_start(out=out_part[1:2, :], in_=CPC[:])
    nc.sync.dma_start(out=out_part[2:3, :], in_=CHR[:])
    nc.sync.dma_start(out=out_part[3:4, :], in_=L1C[:])


# ---------------- host reference partials (for validation) ----------------
def numpy_partials(scores_nc, locs_nc, boxes_nc, priors):
    def cxcy_to_xy(c):
        return np.concatenate([c[..., :2] - c[..., 2:] / 2, c[..., :2] + c[..., 2:] / 2], -1)

    priors_xy = cxcy_to_xy(priors)
    n_pos = np.zeros(C); conf_pos = np.zeros(C); conf_hard = np.zeros(C); l1s = np.zeros(C)
    for c in range(C):
        b = boxes_nc[c]
        lo = np.maximum(b[:, None, :2], priors_xy[None, :, :2])
        hi = np.minimum(b[:, None, 2:], priors_xy[None, :, 2:])
        inter = np.prod(np.clip(hi - lo, 0, None), -1)
        aa = np.prod(b[:, 2:] - b[:, :2], -1)
        ab = np.prod(priors_xy[:, 2:] - priors_xy[:, :2], -1)
        ov = (inter / (aa[:, None] + ab[None, :] - inter)).astype(np.float32)
        ofp = ov.argmax(0); vfp = ov.max(0)
        pfo = ov.argmax(1)
        ofp[pfo] = np.arange(M); vfp[pfo] = 1.0
        pos = vfp >= 0.5
        n_pos[c] = pos.sum()
        d = (scores_nc[c, :, 1] - scores_nc[c, :, 0]).astype(np.float32)
        ce = np.logaddexp(0, np.where(pos, -d, d)).astype(np.float32)
        conf_pos[c] = ce[pos].sum()
        ce_neg = np.where(pos, 0, ce)
        k = int(3 * n_pos[c])
        srt = np.sort(ce_neg)[::-1]
        conf_hard[c] = srt[:k].sum()
        bm = b[ofp]
        bcx = (bm[:, 0] + bm[:, 2]) / 2; bcy = (bm[:, 1] + bm[:, 3]) / 2
        bw = bm[:, 2] - bm[:, 0]; bh = bm[:, 3] - bm[:, 1]
        gcx = (bcx - priors[:, 0]) / (priors[:, 2] / 10)
        gcy = (bcy - priors[:, 1]) / (priors[:, 3] / 10)
        gw = np.log(bw / priors[:, 2]) * 5
        gh = np.log(bh / priors[:, 3]) * 5
        tl = np.stack([gcx, gcy, gw, gh], -1)
        l1 = np.abs(locs_nc[c] - tl).sum(-1) * pos
        l1s[c] = l1.sum()
    return np.stack([n_pos, conf_pos, conf_hard, l1s]).astype(np.float32)


def combine_partials(parts):
    tot = np.sum([p[:4] for p in parts], axis=0).astype(np.float64)
    n_pos_c, conf_pos_c, conf_hard_c, l1_c = tot
    loc_loss_c = l1_c / np.maximum(n_pos_c * 4.0, 1.0)
    safe = np.maximum(n_pos_c, 1.0)
    loss_c = np.where(n_pos_c > 0, (conf_pos_c + conf_hard_c + 1.0 * loc_loss_c) / safe, 0.0) / C
    return np.float32(loss_c.sum())


# ======================= entry point =======================
import os as _os

LAST_EXEC_NS = None
_COMPILED = None
N_CORES = 8


def _install_ntff_hook():
    """Provide antenv.axon_hooks if the image lacks it, so trace=True works."""
    import sys as _sys, types as _types
    try:
        from antenv.axon_hooks import get_axon_ntff_profile_hook  # noqa
        return
    except ImportError:
        pass
    mod = _types.ModuleType("antenv.axon_hooks")
    _h = {"hook": None}
    mod.set_axon_ntff_profile_hook = lambda h: _h.__setitem__("hook", h)
    mod.get_axon_ntff_profile_hook = lambda: _h["hook"]
    _sys.modules["antenv.axon_hooks"] = mod
    try:
        import antenv
        antenv.axon_hooks = mod
        from trn_agent_boot.trn_boot import _ntff_profile_via_ctypes
        mod.set_axon_ntff_profile_hook(_ntff_profile_via_ctypes("/opt/axon/libaxon_pjrt.so"))
    except Exception:
        pass


def _build_module():
    global _COMPILED
    if _COMPILED is not None:
        return _COMPILED
    import concourse.bacc as bacc
    from concourse.bass_interp import get_hw_module

    shapes = {
        "mdcol": (QP, I * CM),
        "scores_pad": (C, QP * 138),
        "locs_pad": (C, QP * 276),
        "boxes_t": (1, C * M * 4),
        "priors_t": (PP, 4),
        "ident": (QP, QP),
        "ind120": (SEL_ROWS, C),
        "indT": (C, SEL_ROWS),
        "later": (QP, QP),
        "coffs": (QP, 3),
        "mvals": (QP, 3),
        "padmask": (QP, IC),
    }
    nc = bacc.Bacc("TRN2", target_bir_lowering=False, debug=False, enable_asserts=False)
    in_aps = {}
    for name, shp in shapes.items():
        t = nc.dram_tensor(name, shp, mybir.dt.float32, kind="ExternalInput")
        in_aps[name] = t.ap()
    out_t = nc.dram_tensor("part", (8, C), mybir.dt.float32, kind="ExternalOutput")
    out_aps = {"part": out_t.ap()}
    with tile.TileContext(nc, trace_sim=False) as tc:
        build_kernel(tc, out_aps, in_aps)
    nc.compile()
    nc.m = get_hw_module(nc.m)
    _COMPILED = nc
    return nc


def kernel(predicted_locs, predicted_scores, boxes, labels, priors_cxcy):
    """Full (unsharded) inputs -> full scalar output. Data-parallel over N on 8 cores."""
    global LAST_EXEC_NS
    from concourse import bass_utils

    predicted_locs = np.ascontiguousarray(predicted_locs, np.float32)
    predicted_scores = np.ascontiguousarray(predicted_scores, np.float32)
    boxes = np.ascontiguousarray(boxes, np.float32)
    priors_cxcy = np.ascontiguousarray(priors_cxcy, np.float32)

    shared = prep_shared_inputs(priors_cxcy)
    in_maps = []
    for n in range(N_CORES):
        m = dict(shared)
        m.update(prep_core_inputs(predicted_scores[n], predicted_locs[n], boxes[n]))
        in_maps.append(m)

    nc = _build_module()
    trace = _os.environ.get("KERNEL_TRACE", "0") == "1"
    if trace:
        _install_ntff_hook()
    res = bass_utils.run_bass_kernel_spmd(
        nc, in_maps, core_ids=list(range(N_CORES)), trace=trace,
    )
    LAST_EXEC_NS = res.exec_time_ns
    parts = [res.results[n]["part"] for n in range(N_CORES)]
    return combine_partials(parts)
